# revision 1
# baseline (speedup 1.0000x reference)
"""HEPOS BART cross-attention Trainium2 kernel.

Shapes (hardcoded): B=2, Tq=1024, Tk=8192, E=1024, H=16, D=64, stride=16,
m = Tk//stride = 512 keys per head.

Sharding: 8 cores = 2 batches x 4 head-groups (4 heads each).
Each core computes, for its batch b and heads hg=[4g..4g+3]:
  QT   = (Wq_hg @ hs_b^T) * scale + bq  -> [256, 1024]   (d-major)
  KgT  = Wk_h @ kvg_h^T                 -> [64, 512] per head
  VgT  = Wv_h @ kvg_h^T -> PE-transpose -> Vg' [512, 65] (ones col -> rowsum)
  ST   = Kg @ Q^T (scoresT)             -> [512, 1024] per head
  ET   = exp(ST)                        (no max-subtraction; scores are O(1))
  OT'  = Vg'^T @ ET                     -> [65, 1024]: rows 0-63 out, row 64 sum
  OT   = OT'[0:64] * (1/OT'[64])        -> outT_all [256, 1024]
  partial = outT_all^T @ WoT_c          -> [1024, 1024]  (row-parallel)
Host sums the 4 partials per batch and adds (bv @ Wo.T + bo).
bk is dropped: a constant shift of every gathered key adds the same value to
every score in a softmax row, which cancels exactly.

All matmuls run as float32r (fp32 data, 1 cycle/row at N=512).
"""

import numpy as np

import concourse.bass as bass
import concourse.bacc as bacc
import concourse.tile as tile
from concourse import library_config, mybir
from concourse.masks import make_identity

B, Tq, Tk, E, H, D = 2, 1024, 8192, 1024, 16, 64
STRIDE = 16
M = Tk // STRIDE          # 512 keys per head
HPC = 4                   # heads per core
NCORES = 8
F32 = mybir.dt.float32
F32R = mybir.dt.float32r


def r(ap):
    """View an SBUF/PSUM AP as float32r for the tensor engine."""
    return ap.bitcast(F32R)


def build_program():
    nc = bacc.Bacc("TRN2", target_bir_lowering=False)

    hsT = nc.dram_tensor("hsT", [8, 128, Tq], F32R, kind="ExternalInput")
    kvgT = nc.dram_tensor("kvgT", [HPC, 8, 128, M], F32R, kind="ExternalInput")
    wqT = nc.dram_tensor("wqT", [8, 128, 256], F32R, kind="ExternalInput")
    bqh = nc.dram_tensor("bqh", [HPC, D, 1], F32, kind="ExternalInput")
    wkT = nc.dram_tensor("wkT", [HPC, 8, 128, D], F32R, kind="ExternalInput")
    wvT = nc.dram_tensor("wvT", [HPC, 8, 128, D], F32R, kind="ExternalInput")
    woT = nc.dram_tensor("woT", [2, 128, E], F32R, kind="ExternalInput")
    out = nc.dram_tensor("out", [8, 128, E], F32, kind="ExternalOutput")

    _dma_engs = [nc.sync, nc.scalar, nc.gpsimd]
    _dma_i = [0]

    def dma(out, in_):
        eng = _dma_engs[_dma_i[0] % len(_dma_engs)]
        _dma_i[0] += 1
        eng.dma_start(out=out, in_=in_)

    with tile.TileContext(nc) as tc:
        with (
            tc.tile_pool(name="consts", bufs=1) as consts,
            tc.tile_pool(name="kvpool", bufs=2) as kvpool,
            tc.tile_pool(name="exppool", bufs=2) as exppool,
            tc.tile_pool(name="kgpool", bufs=1) as kgpool,
            tc.tile_pool(name="vgpool", bufs=2) as vgpool,
            tc.tile_pool(name="rpool", bufs=4) as rpool,
            tc.tile_pool(name="opool", bufs=3) as opool,
            tc.tile_pool(name="ps_a", bufs=2, space="PSUM") as ps_a,
            tc.tile_pool(name="ps_s", bufs=3, space="PSUM") as ps_s,
            tc.tile_pool(name="ps_o", bufs=3, space="PSUM") as ps_o,
        ):
            # ---- persistent SBUF tiles -------------------------------------
            hsT_sb = consts.tile([128, 8 * Tq], F32R)
            wq_sb = consts.tile([128, 8 * 256], F32R)
            wk_sb = consts.tile([128, HPC * 8 * D], F32R)
            wv_sb = consts.tile([128, HPC * 8 * D], F32R)
            wo_sb = consts.tile([128, 2 * E], F32R)
            ident = consts.tile([128, 128], F32)
            qt_sb = [consts.tile([D, Tq], F32R, name=f"qt{h}") for h in range(HPC)]
            outT_sb = [consts.tile([128, Tq], F32R, name=f"outT{dd}") for dd in range(2)]

            make_identity(nc, ident)
            ones_f = consts.tile([1, 512], F32)
            nc.vector.memset(ones_f[:], 1.0)
            ones_sb = consts.tile([1, 512], F32R)
            nc.vector.tensor_copy(ones_sb[:], ones_f[:])
            onescol_f = consts.tile([128, HPC, 1], F32)
            nc.vector.memset(onescol_f[:], 1.0)

            # ---- input DMAs ------------------------------------------------
            for e in range(8):
                dma(out=wq_sb[:, e * 256:(e + 1) * 256], in_=wqT[e])
            for e in range(8):
                dma(out=hsT_sb[:, e * Tq:(e + 1) * Tq], in_=hsT[e])
            for h in range(HPC):
                for e in range(8):
                    dma(
                        out=wk_sb[:, (h * 8 + e) * D:(h * 8 + e + 1) * D],
                        in_=wkT[h, e])
                    dma(
                        out=wv_sb[:, (h * 8 + e) * D:(h * 8 + e + 1) * D],
                        in_=wvT[h, e])
            for dd in range(2):
                dma(out=wo_sb[:, dd * E:(dd + 1) * E], in_=woT[dd])

            bq_tiles = [consts.tile([D, 1], F32, name=f"bq{h}") for h in range(HPC)]
            for h in range(HPC):
                dma(out=bq_tiles[h][:], in_=bqh[h])

            # ---- phase 1: QT projection ------------------------------------
            # psum [128, 512] holds a head pair (rows 0-63 head 2p, 64-127 head 2p+1)
            for pair in range(2):
                for tqt in range(2):
                    ps_qt = ps_s.tile([128, 512], F32, tag="ps_s")
                    for e in range(8):
                        nc.tensor.matmul(
                            ps_qt[:],
                            r(wq_sb[:, e * 256 + pair * 128: e * 256 + (pair + 1) * 128]),
                            r(hsT_sb[:, e * Tq + tqt * 512: e * Tq + tqt * 512 + 512]),
                            start=(e == 0), stop=(e == 7))
                    for sub in range(2):
                        h = 2 * pair + sub
                        nc.scalar.activation(
                            qt_sb[h][:, tqt * 512: tqt * 512 + 512],
                            ps_qt[sub * 64:(sub + 1) * 64, :],
                            mybir.ActivationFunctionType.Identity,
                            bias=bq_tiles[h][:])

            # ---- phase 2: per-head K/V proj + attention --------------------
            for h in range(HPC):
                kvg_sb = kvpool.tile([128, 8 * M], F32R, tag="kvg")
                for e in range(8):
                    dma(
                        out=kvg_sb[:, e * M:(e + 1) * M], in_=kvgT[h, e])

                # K^T_g [64, 512]
                kg_sb = kgpool.tile([D, M], F32R, tag="kg", bufs=2)
                ps_kg = ps_a.tile([D, M], F32, tag="ps_a")
                for e in range(8):
                    nc.tensor.matmul(
                        ps_kg[:],
                        r(wk_sb[:, (h * 8 + e) * D:(h * 8 + e + 1) * D]),
                        r(kvg_sb[:, e * M:(e + 1) * M]),
                        start=(e == 0), stop=(e == 7))
                nc.vector.tensor_copy(kg_sb[:], ps_kg[:])

                # V^T_g [64, 512] -> transpose into Vg' [4][128, 65]
                vgT_sb = vgpool.tile([D, M], F32, tag="vgT")
                ps_vg = ps_a.tile([D, M], F32, tag="ps_a")
                for e in range(8):
                    nc.tensor.matmul(
                        ps_vg[:],
                        r(wv_sb[:, (h * 8 + e) * D:(h * 8 + e + 1) * D]),
                        r(kvg_sb[:, e * M:(e + 1) * M]),
                        start=(e == 0), stop=(e == 7))
                nc.vector.tensor_copy(vgT_sb[:], ps_vg[:])

                vgp_sb = vgpool.tile([128, 4, D + 1], F32R, tag="vgp")
                nc.vector.tensor_copy(vgp_sb[:, :, D:D + 1], onescol_f[:])
                for mc in range(4):
                    ps_vt = ps_a.tile([128, D], F32, tag="ps_a")
                    nc.tensor.transpose(
                        ps_vt[:], vgT_sb[:, mc * 128:(mc + 1) * 128],
                        ident[0:D, 0:D])
                    nc.vector.tensor_copy(vgp_sb[:, mc, 0:D], ps_vt[:])

                # scoresT -> exp -> attnT @ Vg' -> normalize
                expT_sb = exppool.tile([128, 4, Tq], F32R, tag="expT")
                for tqt in range(2):
                    for mc in range(4):
                        ps_sc = ps_s.tile([128, 512], F32, tag="ps_s")
                        nc.tensor.matmul(
                            ps_sc[:],
                            r(kg_sb[:, mc * 128:(mc + 1) * 128]),
                            r(qt_sb[h][:, tqt * 512: tqt * 512 + 512]),
                            start=True, stop=True)
                        nc.scalar.activation(
                            expT_sb[:, mc, tqt * 512: tqt * 512 + 512],
                            ps_sc[:],
                            mybir.ActivationFunctionType.Exp)

                    ps_ov = ps_o.tile([128, 512], F32, tag="ps_o")
                    for mc in range(4):
                        nc.tensor.matmul(
                            ps_ov[0:D + 1, :],
                            r(vgp_sb[:, mc, :]),
                            r(expT_sb[:, mc, tqt * 512: tqt * 512 + 512]),
                            start=(mc == 0), stop=(mc == 3))
                    rinv = rpool.tile([1, 512], F32R, tag="rinv")
                    with nc.allow_low_precision(reason="fp32r feed for PE bcast"):
                        nc.vector.reciprocal(rinv[:], ps_ov[D:D + 1, :])
                    ps_rb = ps_a.tile([D, 512], F32, tag="ps_a")
                    nc.tensor.matmul(ps_rb[:], ones_sb[:, 0:D], rinv[:],
                                     start=True, stop=True)
                    rinv_b = rpool.tile([D, 512], F32, tag="rinv_b")
                    nc.vector.tensor_copy(rinv_b[:], ps_rb[:])
                    nc.vector.tensor_tensor(
                        outT_sb[h // 2][(h % 2) * D:(h % 2 + 1) * D,
                                        tqt * 512: tqt * 512 + 512],
                        ps_ov[0:D, :],
                        rinv_b[:],
                        op=mybir.AluOpType.mult)

            # ---- phase 3: out-projection (row-parallel partial) ------------
            for t8 in range(8):
                for eot in range(2):
                    ps_po = ps_o.tile([128, 512], F32, tag="ps_o")
                    for dd in range(2):
                        nc.tensor.matmul(
                            ps_po[:],
                            r(outT_sb[dd][:, t8 * 128:(t8 + 1) * 128]),
                            r(wo_sb[:, dd * E + eot * 512: dd * E + eot * 512 + 512]),
                            start=(dd == 0), stop=(dd == 1))
                    po_sb = opool.tile([128, 512], F32, tag="po")
                    nc.vector.tensor_copy(po_sb[:], ps_po[:])
                    dma(
                        out=out[t8][:, eot * 512: eot * 512 + 512], in_=po_sb[:])

    nc.compile()
    return nc


_NC = None


def _get_nc():
    global _NC
    if _NC is None:
        _NC = build_program()
    return _NC


def round_fp32r(a):
    """Round-to-nearest-even to fp32r: 11-bit mantissa, low 12 bits zero."""
    u = np.ascontiguousarray(a, np.float32).view(np.uint32)
    u = u + 0x7FF + ((u >> 12) & 1)
    u &= np.uint32(0xFFFFF000)
    return u.view(np.float32)


def shard_inputs(hidden_states, key_value_states, Wq, bq, Wk, bk, Wv, bv, Wo, bo,
                 stride):
    stride = int(stride)
    assert stride == STRIDE
    scale = float(D) ** -0.5
    in_maps = []
    for c in range(NCORES):
        b, g = divmod(c, 4)
        h0 = g * HPC  # first global head of this core
        r0, r1 = h0 * D, (h0 + HPC) * D
        hsT_c = np.ascontiguousarray(hidden_states[b].T).reshape(8, 128, Tq)
        kvgT_c = np.empty((HPC, 8, 128, M), np.float32)
        for hl in range(HPC):
            rows = key_value_states[b, (h0 + hl)::STRIDE, :]  # [M, E]
            kvgT_c[hl] = np.ascontiguousarray(rows.T).reshape(8, 128, M)
        wqT_c = np.ascontiguousarray((Wq[r0:r1, :] * scale).T).reshape(8, 128, 256)
        bq_c = (bq[r0:r1] * scale).astype(np.float32).reshape(HPC, D, 1)
        wkT_c = np.ascontiguousarray(
            Wk[r0:r1, :].reshape(HPC, D, E).transpose(0, 2, 1)).reshape(
                HPC, 8, 128, D)
        wvT_c = np.ascontiguousarray(
            Wv[r0:r1, :].reshape(HPC, D, E).transpose(0, 2, 1)).reshape(
                HPC, 8, 128, D)
        woT_c = np.ascontiguousarray(Wo[:, r0:r1].T).reshape(2, 128, E)
        in_maps.append({
            "hsT": round_fp32r(hsT_c),
            "kvgT": round_fp32r(kvgT_c),
            "wqT": round_fp32r(wqT_c),
            "bqh": bq_c,
            "wkT": round_fp32r(wkT_c),
            "wvT": round_fp32r(wvT_c),
            "woT": round_fp32r(woT_c),
        })
    return in_maps


def combine_outputs(results, Wv, bv, Wo, bo):
    final_bias = (bv @ Wo.T + bo).astype(np.float32)  # [E]
    out = np.zeros((B, Tq, E), np.float32)
    for c in range(NCORES):
        b = c // 4
        out[b] += results[c]["out"].reshape(Tq, E)
    out += final_bias[None, None, :]
    return out


def kernel(hidden_states, key_value_states, Wq, bq, Wk, bk, Wv, bv, Wo, bo,
           stride, _trace=False, _trace_kwargs=None):
    from concourse.bass_utils import run_bass_kernel_spmd

    args = [np.asarray(x, np.float32) for x in
            (hidden_states, key_value_states, Wq, bq, Wk, bk, Wv, bv, Wo, bo)]
    (hidden_states, key_value_states, Wq, bq, Wk, bk, Wv, bv, Wo, bo) = args
    in_maps = shard_inputs(hidden_states, key_value_states, Wq, bq, Wk, bk,
                           Wv, bv, Wo, bo, stride)
    nc = _get_nc()
    res = run_bass_kernel_spmd(
        nc, in_maps, list(range(NCORES)),
        trace=_trace, **(_trace_kwargs or {}))
    out = combine_outputs(res.results, Wv, bv, Wo, bo)
    kernel.last_run = res
    return out



# revision 11
# speedup vs baseline: 1.7029x; 1.7029x over previous
"""HEPOS BART cross-attention Trainium2 kernel (bf16, PE-dense rewrite).

Shapes (hardcoded): B=2, Tq=1024, Tk=8192, E=1024, H=16, D=64, stride=16,
m = Tk//stride = 512 keys per head.

Sharding: 8 cores = 2 batches x 4 head-groups (4 heads each).
All DMA'd tensors are bf16 (halves HBM traffic vs fp32; PE rate is the same
1 cycle/row as fp32r). PSUM accumulation is fp32 throughout.

Per core:
  phase 1: QT = (Wq_hg @ hs_b^T)*scale + bq  -> qt[h] [64, 1024] bf16
           (4 sequential psum groups of 8 matmuls; bias added on DVE)
  phase 2a per head: fused KV proj: stationary [Wk_h | Wv_h] chunk [128,128]
           -> psum [128, 512]: rows 0-63 KgT, 64-127 VgT. One DVE copy to
           SBUF; 4 PE transposes of the V half -> vgp [128, 4, 65]
           (col 64 = ones -> softmax denominator comes free in attn@V).
  phase 2b (tqt outer, h inner):
           scores: 4 matmuls into 2x [128, 2, 512] psum tiles
           exp: 2 scalar activations [128, 1024] -> expT bf16
           attn@V: 4 matmuls accum -> ps_ov [65, 512] (row 64 = Z)
           rinv = 1/Z: even heads on scalar as exp(-ln(Z)) (ln+exp share one
           ACT table -> no table thrash), odd heads on DVE reciprocal.
           PE broadcast ones x rinv -> ps_rb; DVE mult -> outT bf16.
  phase 3 (per tqt half): partial = outT^T @ WoT -> psum -> SBUF -> DMA.
bk is dropped (constant key shift cancels in softmax); bv folded into the
host-side final bias (bv @ Wo.T + bo).
"""

import numpy as np
import ml_dtypes

import concourse.bass as bass
import concourse.bacc as bacc
import concourse.tile as tile
from concourse import library_config, mybir
from concourse.masks import make_identity

B, Tq, Tk, E, H, D = 2, 1024, 8192, 1024, 16, 64
STRIDE = 16
M = Tk // STRIDE          # 512 keys per head
HPC = 4                   # heads per core
NCORES = 8
F32 = mybir.dt.float32
BF16 = mybir.dt.bfloat16
NPBF16 = ml_dtypes.bfloat16


def build_program():
    nc = bacc.Bacc("TRN2", target_bir_lowering=False)

    # dram tensors already in SBUF layout
    hsT = nc.dram_tensor("hsT", [128, 8 * Tq], BF16, kind="ExternalInput")
    wq = nc.dram_tensor("wq", [128, 2048], BF16, kind="ExternalInput")
    wkv = nc.dram_tensor("wkv", [128, 4096], BF16, kind="ExternalInput")
    kvg = nc.dram_tensor("kvg", [HPC, 128, 8 * M], BF16, kind="ExternalInput")
    wo = nc.dram_tensor("wo", [128, 2048], BF16, kind="ExternalInput")
    bqp = nc.dram_tensor("bqp", [2, 128, 1], F32, kind="ExternalInput")
    out = nc.dram_tensor("out", [16, 128, 512], BF16, kind="ExternalOutput")

    _dma_engs = [None, None]
    _dma_i = [0]

    def dma(out_ap, in_ap):
        eng = _dma_engs[_dma_i[0] % 2]
        _dma_i[0] += 1
        eng.dma_start(out=out_ap, in_=in_ap)

    with tile.TileContext(nc) as tc:
        _dma_engs[0] = nc.sync
        _dma_engs[1] = nc.gpsimd
        with (
            tc.tile_pool(name="consts", bufs=1) as consts,
            tc.tile_pool(name="expp", bufs=2) as expp,
            tc.tile_pool(name="rowp", bufs=2) as rowp,
            tc.tile_pool(name="pop", bufs=3) as pop,
            tc.tile_pool(name="psA", bufs=2, space="PSUM") as psA,
            tc.tile_pool(name="psB", bufs=4, space="PSUM") as psB,
        ):
            # ---- persistent SBUF tiles -------------------------------------
            hsT_sb = consts.tile([128, 8 * Tq], BF16)
            wq_sb = consts.tile([128, 2048], BF16)
            wkv_sb = consts.tile([128, 4096], BF16)
            wo_sb = consts.tile([128, 2048], BF16)
            kvg_sb = [consts.tile([128, 8 * M], BF16, name=f"kvg{h}")
                      for h in range(HPC)]
            kv_sb = [consts.tile([128, M], BF16, name=f"kv{h}")
                     for h in range(HPC)]
            vgp_sb = [consts.tile([128, 4, D + 1], BF16, name=f"vgp{h}")
                      for h in range(HPC)]
            qt_sb = [consts.tile([D, Tq], BF16, name=f"qt{h}")
                     for h in range(HPC)]
            outT_sb = [consts.tile([128, Tq], BF16, name=f"outT{dd}")
                       for dd in range(2)]
            bq_sb = [consts.tile([128, 1], F32, name=f"bq{p}") for p in range(2)]

            identf = consts.tile([128, 128], F32)
            make_identity(nc, identf)
            ident = consts.tile([128, 128], BF16)
            nc.vector.tensor_copy(ident[:], identf[:])
            for h in range(HPC):
                nc.vector.memset(vgp_sb[h][:, :, D:D + 1], 1.0)

            # ---- input DMAs (interleaved sync/gpsimd, priority order) ------
            dma(wq_sb[:], wq[:])                                   # sync
            dma(hsT_sb[:, 0:2048], hsT[:, 0:2048])                 # gpsimd
            dma(hsT_sb[:, 2048:4096], hsT[:, 2048:4096])           # sync
            dma(kvg_sb[0][:, 0:2048], kvg[0][:, 0:2048])           # gpsimd
            dma(hsT_sb[:, 4096:6144], hsT[:, 4096:6144])           # sync
            dma(kvg_sb[0][:, 2048:4096], kvg[0][:, 2048:4096])     # gpsimd
            dma(hsT_sb[:, 6144:8192], hsT[:, 6144:8192])           # sync
            dma(wkv_sb[:, 0:1024], wkv[:, 0:1024])                 # gpsimd
            dma(kvg_sb[1][:, 0:2048], kvg[1][:, 0:2048])
            dma(kvg_sb[1][:, 2048:4096], kvg[1][:, 2048:4096])
            dma(wkv_sb[:, 1024:2048], wkv[:, 1024:2048])
            dma(bq_sb[0][:], bqp[0])
            dma(bq_sb[1][:], bqp[1])
            dma(kvg_sb[2][:, 0:2048], kvg[2][:, 0:2048])
            dma(kvg_sb[2][:, 2048:4096], kvg[2][:, 2048:4096])
            dma(wkv_sb[:, 2048:3072], wkv[:, 2048:3072])
            dma(kvg_sb[3][:, 0:2048], kvg[3][:, 0:2048])
            dma(kvg_sb[3][:, 2048:4096], kvg[3][:, 2048:4096])
            dma(wkv_sb[:, 3072:4096], wkv[:, 3072:4096])
            dma(wo_sb[:, 0:1024], wo[:, 0:1024])
            dma(wo_sb[:, 1024:2048], wo[:, 1024:2048])

            # ---- phase 1: QT projection ------------------------------------
            # 4 sequential psum groups (pair, tqt); accumulate over 8 e-chunks
            for pair in range(2):
                for tqt in range(2):
                    ps_qt = psA.tile([128, 2, 512], F32, tag="A", name="ps_qt")
                    for e in range(8):
                        nc.tensor.matmul(
                            ps_qt[:, 0, :],
                            wq_sb[:, e * 256 + pair * 128:
                                  e * 256 + (pair + 1) * 128],
                            hsT_sb[:, e * Tq + tqt * 512:
                                   e * Tq + tqt * 512 + 512],
                            start=(e == 0), stop=(e == 7))
                    for sub in range(2):
                        h = 2 * pair + sub
                        if sub == 0:
                            nc.vector.tensor_scalar_add(
                                qt_sb[h][:, tqt * 512: tqt * 512 + 512],
                                ps_qt[0:64, 0, :],
                                bq_sb[pair][0:64, 0:1])
                        else:
                            nc.scalar.activation(
                                qt_sb[h][:, tqt * 512: tqt * 512 + 512],
                                ps_qt[64:128, 0, :],
                                mybir.ActivationFunctionType.Identity,
                                bias=bq_sb[pair][64:128, 0:1])

            # ---- phase 2a: fused KV projection + V transpose ---------------
            for h in range(HPC):
                ps_kv = psA.tile([128, 2, 512], F32, tag="A", name="ps_kv")
                for e in range(8):
                    nc.tensor.matmul(
                        ps_kv[:, 0, :],
                        wkv_sb[:, (h * 8 + e) * 128:(h * 8 + e + 1) * 128],
                        kvg_sb[h][:, e * M:(e + 1) * M],
                        start=(e == 0), stop=(e == 7))
                if h % 2 == 0:
                    nc.vector.tensor_copy(kv_sb[h][:], ps_kv[:, 0, :])
                else:
                    nc.scalar.copy(kv_sb[h][:], ps_kv[:, 0, :])
                ps_vt = psB.tile([128, 4, D], BF16, tag="B", name="ps_vt")
                for mc in range(4):
                    nc.tensor.transpose(
                        ps_vt[:, mc, :],
                        kv_sb[h][64:128, mc * 128:(mc + 1) * 128],
                        ident[64:128, 64:128])
                if h % 2 == 0:
                    nc.scalar.copy(vgp_sb[h][:, :, 0:D], ps_vt[:])
                else:
                    nc.vector.tensor_copy(vgp_sb[h][:, :, 0:D], ps_vt[:])

            # ---- phase 2b + 3: attention, then row-parallel out-proj -------
            for tqt in range(2):
                for h in range(HPC):
                    # scores -> exp
                    expT = expp.tile([128, 4, 512], BF16, tag="expT",
                                     name="expT")
                    for mcp in range(2):
                        ps_sc = psA.tile([128, 2, 512], F32, tag="A",
                                         name="ps_sc")
                        for sub in range(2):
                            mc = 2 * mcp + sub
                            nc.tensor.matmul(
                                ps_sc[:, sub, :],
                                kv_sb[h][0:D, mc * 128:(mc + 1) * 128],
                                qt_sb[h][:, tqt * 512: tqt * 512 + 512],
                                start=True, stop=True)
                        nc.scalar.activation(
                            expT[:, 2 * mcp:2 * mcp + 2, :],
                            ps_sc[:],
                            mybir.ActivationFunctionType.Exp)
                    # attn @ V (+ denominator in row 64)
                    ps_ov = psB.tile([128, 512], F32, tag="B", name="ps_ov")
                    for mc in range(4):
                        nc.tensor.matmul(
                            ps_ov[0:D + 1, :],
                            vgp_sb[h][:, mc, :],
                            expT[:, mc, :],
                            start=(mc == 0), stop=(mc == 3))
                    # rinv = 1/Z on DVE (fast approx), broadcast on GpSimd.
                    # Z must reach SBUF partition 0 via a native op first: the
                    # custom-DVE recip misreads partition-shifted PSUM on HW.
                    zrow = rowp.tile([1, 512], F32, tag="zrow", name="zrow")
                    if h % 2 == 0:
                        nc.scalar.copy(zrow[:], ps_ov[D:D + 1, :])
                    else:
                        nc.vector.tensor_copy(zrow[:], ps_ov[D:D + 1, :])
                    rinv = rowp.tile([1, 512], F32, tag="rinv", name="rinv")
                    nc.vector.reciprocal_approx_fast(rinv[:], zrow[:])
                    rinv_b = rowp.tile([D, 512], F32, tag="rinv_b",
                                       name="rinv_b")
                    nc.gpsimd.partition_broadcast(rinv_b[:], rinv[:])
                    nc.vector.tensor_tensor(
                        outT_sb[h // 2][(h % 2) * D:(h % 2 + 1) * D,
                                        tqt * 512: tqt * 512 + 512],
                        ps_ov[0:D, :],
                        rinv_b[:],
                        op=mybir.AluOpType.mult)

                # out-projection for this tqt half (t8 chunks)
                for t8 in range(4 * tqt, 4 * tqt + 4):
                    ps_po = psA.tile([128, 2, 512], F32, tag="A",
                                     name="ps_po")
                    for eot in range(2):
                        for dd in range(2):
                            nc.tensor.matmul(
                                ps_po[:, eot, :],
                                outT_sb[dd][:, t8 * 128:(t8 + 1) * 128],
                                wo_sb[:, dd * E + eot * 512:
                                      dd * E + eot * 512 + 512],
                                start=(dd == 0), stop=(dd == 1))
                    po_sb = pop.tile([128, 2, 512], BF16, tag="po",
                                     name="po_sb")
                    if t8 % 2 == 0:
                        nc.scalar.copy(po_sb[:], ps_po[:])
                    else:
                        nc.vector.tensor_copy(po_sb[:], ps_po[:])
                    dma(out[t8 * 2], po_sb[:, 0, :])
                    dma(out[t8 * 2 + 1], po_sb[:, 1, :])

    nc.compile()
    return nc


_NC = None


def _get_nc():
    global _NC
    if _NC is None:
        _NC = build_program()
    return _NC


def shard_inputs(hidden_states, key_value_states, Wq, bq, Wk, bk, Wv, bv, Wo,
                 bo, stride):
    stride = int(stride)
    assert stride == STRIDE
    scale = float(D) ** -0.5
    bf = lambda a: np.ascontiguousarray(a).astype(NPBF16)
    in_maps = []
    for c in range(NCORES):
        b, g = divmod(c, 4)
        h0 = g * HPC
        r0, r1 = h0 * D, (h0 + HPC) * D
        # hsT [128, 8*1024]: col e*1024+t = hs[b][t, e*128+p]
        hsT_c = hidden_states[b].T.reshape(8, 128, Tq).transpose(1, 0, 2) \
            .reshape(128, 8 * Tq)
        # wq [128, 2048]: col e*256+pair*128+j = Wq_s[pair*128+j, e*128+p]
        Wqs = (Wq[r0:r1, :] * scale)
        wq_c = Wqs.T.reshape(8, 128, 2, 128).transpose(1, 0, 2, 3) \
            .reshape(128, 2048)
        # wkv [128, 4096]: col (h*8+e)*128+j: j<64 Wk, j>=64 Wv
        K3 = Wk[r0:r1, :].reshape(HPC, D, E)
        V3 = Wv[r0:r1, :].reshape(HPC, D, E)
        C = np.concatenate([K3, V3], axis=1)          # [4, 128, 1024]
        wkv_c = C.reshape(4, 128, 8, 128).transpose(3, 0, 2, 1) \
            .reshape(128, 4096)
        # kvg [4, 128, 4096]: col e*512+m = kv[b, h0+h+16m, e*128+p]
        kvg_c = np.empty((HPC, 128, 8 * M), NPBF16)
        for hl in range(HPC):
            R = key_value_states[b, (h0 + hl)::STRIDE, :]     # [512, 1024]
            kvg_c[hl] = bf(R.T.reshape(8, 128, M).transpose(1, 0, 2)
                           .reshape(128, 8 * M))
        # wo [128, 2048]: col dd*1024+n = Wo[n, r0+dd*128+p]
        wo_c = Wo[:, r0:r1].T.reshape(2, 128, E).transpose(1, 0, 2) \
            .reshape(128, 2048)
        bqp_c = (bq[r0:r1] * scale).astype(np.float32).reshape(2, 128, 1)
        in_maps.append({
            "hsT": bf(hsT_c),
            "wq": bf(wq_c),
            "wkv": bf(wkv_c),
            "kvg": np.ascontiguousarray(kvg_c),
            "wo": bf(wo_c),
            "bqp": bqp_c,
        })
    return in_maps


def combine_outputs(results, Wv, bv, Wo, bo):
    final_bias = (bv @ Wo.T + bo).astype(np.float32)  # [E]
    out = np.zeros((B, Tq, E), np.float32)
    for c in range(NCORES):
        b = c // 4
        # out dram [16, 128, 512]: chunk t8*2+eot
        o = results[c]["out"].astype(np.float32) \
            .reshape(8, 2, 128, 512).transpose(0, 2, 1, 3).reshape(Tq, E)
        out[b] += o
    out += final_bias[None, None, :]
    return out


def kernel(hidden_states, key_value_states, Wq, bq, Wk, bk, Wv, bv, Wo, bo,
           stride, _trace=False, _trace_kwargs=None):
    from concourse.bass_utils import run_bass_kernel_spmd

    args = [np.asarray(x, np.float32) for x in
            (hidden_states, key_value_states, Wq, bq, Wk, bk, Wv, bv, Wo, bo)]
    (hidden_states, key_value_states, Wq, bq, Wk, bk, Wv, bv, Wo, bo) = args
    in_maps = shard_inputs(hidden_states, key_value_states, Wq, bq, Wk, bk,
                           Wv, bv, Wo, bo, stride)
    nc = _get_nc()
    res = run_bass_kernel_spmd(
        nc, in_maps, list(range(NCORES)),
        trace=_trace, **(_trace_kwargs or {}))
    out = combine_outputs(res.results, Wv, bv, Wo, bo)
    kernel.last_run = res
    return out


# revision 14
# speedup vs baseline: 1.8281x; 1.0735x over previous
"""HEPOS BART cross-attention Trainium2 kernel (bf16, PE-dense rewrite).

Shapes (hardcoded): B=2, Tq=1024, Tk=8192, E=1024, H=16, D=64, stride=16,
m = Tk//stride = 512 keys per head.

Sharding: 8 cores = 2 batches x 4 head-groups (4 heads each).
All DMA'd tensors are bf16 (halves HBM traffic vs fp32; PE rate is the same
1 cycle/row as fp32r). PSUM accumulation is fp32 throughout.

Per core:
  phase 1: QT = (Wq_hg @ hs_b^T)*scale + bq  -> qt[h] [64, 1024] bf16
           (4 sequential psum groups of 8 matmuls; bias added on DVE)
  phase 2a per head: fused KV proj: stationary [Wk_h | Wv_h] chunk [128,128]
           -> psum [128, 512]: rows 0-63 KgT, 64-127 VgT. One DVE copy to
           SBUF; 4 PE transposes of the V half -> vgp [128, 4, 65]
           (col 64 = ones -> softmax denominator comes free in attn@V).
  phase 2b (tqt outer, h inner):
           scores: 4 matmuls into 2x [128, 2, 512] psum tiles
           exp: 2 scalar activations [128, 1024] -> expT bf16
           attn@V: 4 matmuls accum -> ps_ov [65, 512] (row 64 = Z)
           rinv = 1/Z: even heads on scalar as exp(-ln(Z)) (ln+exp share one
           ACT table -> no table thrash), odd heads on DVE reciprocal.
           PE broadcast ones x rinv -> ps_rb; DVE mult -> outT bf16.
  phase 3 (per tqt half): partial = outT^T @ WoT -> psum -> SBUF -> DMA.
bk is dropped (constant key shift cancels in softmax); bv folded into the
host-side final bias (bv @ Wo.T + bo).
"""

import numpy as np
import ml_dtypes

import concourse.bass as bass
import concourse.bacc as bacc
import concourse.tile as tile
from concourse import library_config, mybir
from concourse.masks import make_identity

B, Tq, Tk, E, H, D = 2, 1024, 8192, 1024, 16, 64
STRIDE = 16
M = Tk // STRIDE          # 512 keys per head
HPC = 4                   # heads per core
NCORES = 8
F32 = mybir.dt.float32
BF16 = mybir.dt.bfloat16
NPBF16 = ml_dtypes.bfloat16


def build_program():
    nc = bacc.Bacc("TRN2", target_bir_lowering=False)

    # dram tensors already in SBUF layout
    # hsT col = tqt*4096 + e*512 + t ; wq col = pair*1024 + e*128 + j
    hsT = nc.dram_tensor("hsT", [128, 8 * Tq], BF16, kind="ExternalInput")
    wq = nc.dram_tensor("wq", [128, 2048], BF16, kind="ExternalInput")
    wkv = nc.dram_tensor("wkv", [128, 4096], BF16, kind="ExternalInput")
    kvg = nc.dram_tensor("kvg", [HPC, 128, 8 * M], BF16, kind="ExternalInput")
    wo = nc.dram_tensor("wo", [128, 2048], BF16, kind="ExternalInput")
    bqp = nc.dram_tensor("bqp", [2, 128, 1], F32, kind="ExternalInput")
    out = nc.dram_tensor("out", [16, 128, 512], BF16, kind="ExternalOutput")

    _dma_engs = [None, None]
    _dma_i = [0]

    def dma(out_ap, in_ap):
        eng = _dma_engs[_dma_i[0] % 2]
        _dma_i[0] += 1
        eng.dma_start(out=out_ap, in_=in_ap)

    with tile.TileContext(nc) as tc:
        _dma_engs[0] = nc.sync
        _dma_engs[1] = nc.gpsimd
        with (
            tc.tile_pool(name="consts", bufs=1) as consts,
            tc.tile_pool(name="expp", bufs=2) as expp,
            tc.tile_pool(name="rowp", bufs=2) as rowp,
            tc.tile_pool(name="pop", bufs=3) as pop,
            tc.tile_pool(name="psA", bufs=2, space="PSUM") as psA,
            tc.tile_pool(name="psB", bufs=4, space="PSUM") as psB,
        ):
            # ---- persistent SBUF tiles -------------------------------------
            hsT_sb = consts.tile([128, 8 * Tq], BF16)
            wq_sb = consts.tile([128, 2048], BF16)
            wkv_sb = consts.tile([128, 4096], BF16)
            wo_sb = consts.tile([128, 2048], BF16)
            kvg_sb = [consts.tile([128, 8 * M], BF16, name=f"kvg{h}")
                      for h in range(HPC)]
            kv_sb = [consts.tile([128, M], BF16, name=f"kv{h}")
                     for h in range(HPC)]
            vgp_sb = [consts.tile([128, 4, D + 1], BF16, name=f"vgp{h}")
                      for h in range(HPC)]
            qt_sb = [consts.tile([D, Tq], BF16, name=f"qt{h}")
                     for h in range(HPC)]
            outT_sb = [consts.tile([128, Tq], BF16, name=f"outT{dd}")
                       for dd in range(2)]
            bq_sb = [consts.tile([128, 1], F32, name=f"bq{p}") for p in range(2)]

            identf = consts.tile([128, 128], F32)
            make_identity(nc, identf)
            ident = consts.tile([128, 128], BF16)
            nc.vector.tensor_copy(ident[:], identf[:])
            for h in range(HPC):
                nc.vector.memset(vgp_sb[h][:, :, D:D + 1], 1.0)

            # ---- input DMAs (interleaved sync/gpsimd, priority order) ------
            dma(wq_sb[:, 0:1024], wq[:, 0:1024])                   # sync
            dma(hsT_sb[:, 0:2048], hsT[:, 0:2048])                 # gpsimd
            dma(hsT_sb[:, 2048:4096], hsT[:, 2048:4096])           # sync
            dma(wq_sb[:, 1024:2048], wq[:, 1024:2048])             # gpsimd
            dma(kvg_sb[0][:, 0:2048], kvg[0][:, 0:2048])           # sync
            dma(kvg_sb[0][:, 2048:4096], kvg[0][:, 2048:4096])     # gpsimd
            dma(hsT_sb[:, 4096:6144], hsT[:, 4096:6144])           # sync
            dma(hsT_sb[:, 6144:8192], hsT[:, 6144:8192])           # gpsimd
            dma(wkv_sb[:, 0:1024], wkv[:, 0:1024])                 # sync
            dma(bq_sb[0][:], bqp[0])
            dma(bq_sb[1][:], bqp[1])
            dma(kvg_sb[1][:, 0:2048], kvg[1][:, 0:2048])
            dma(kvg_sb[1][:, 2048:4096], kvg[1][:, 2048:4096])
            dma(wkv_sb[:, 1024:2048], wkv[:, 1024:2048])
            dma(kvg_sb[2][:, 0:2048], kvg[2][:, 0:2048])
            dma(kvg_sb[2][:, 2048:4096], kvg[2][:, 2048:4096])
            dma(wkv_sb[:, 2048:3072], wkv[:, 2048:3072])
            dma(kvg_sb[3][:, 0:2048], kvg[3][:, 0:2048])
            dma(kvg_sb[3][:, 2048:4096], kvg[3][:, 2048:4096])
            dma(wkv_sb[:, 3072:4096], wkv[:, 3072:4096])
            dma(wo_sb[:, 0:1024], wo[:, 0:1024])
            dma(wo_sb[:, 1024:2048], wo[:, 1024:2048])

            # ---- stage closures (issued in an interleaved order so the
            # in-order PE queue never head-of-line blocks on a late DMA) ----
            def ph1(pair, tqt):
                ps_qt = psA.tile([128, 2, 512], F32, tag="A", name="ps_qt")
                for e in range(8):
                    nc.tensor.matmul(
                        ps_qt[:, 0, :],
                        wq_sb[:, pair * 1024 + e * 128:
                              pair * 1024 + (e + 1) * 128],
                        hsT_sb[:, tqt * 4096 + e * 512:
                               tqt * 4096 + (e + 1) * 512],
                        start=(e == 0), stop=(e == 7))
                for sub in range(2):
                    h = 2 * pair + sub
                    nc.scalar.activation(
                        qt_sb[h][:, tqt * 512: tqt * 512 + 512],
                        ps_qt[sub * 64:(sub + 1) * 64, 0, :],
                        mybir.ActivationFunctionType.Identity,
                        bias=bq_sb[pair][sub * 64:(sub + 1) * 64, 0:1])

            def p2a(h):
                ps_kv = psA.tile([128, 2, 512], F32, tag="A", name="ps_kv")
                for e in range(8):
                    nc.tensor.matmul(
                        ps_kv[:, 0, :],
                        wkv_sb[:, (h * 8 + e) * 128:(h * 8 + e + 1) * 128],
                        kvg_sb[h][:, e * M:(e + 1) * M],
                        start=(e == 0), stop=(e == 7))
                nc.scalar.copy(kv_sb[h][:], ps_kv[:, 0, :])
                ps_vt = psB.tile([128, 4, D], BF16, tag="B", name="ps_vt")
                for mc in range(4):
                    nc.tensor.transpose(
                        ps_vt[:, mc, :],
                        kv_sb[h][64:128, mc * 128:(mc + 1) * 128],
                        ident[64:128, 64:128])
                nc.vector.tensor_copy(vgp_sb[h][:, :, 0:D], ps_vt[:])

            def att(tqt, h):
                expT = expp.tile([128, 4, 512], BF16, tag="expT", name="expT")
                for mcp in range(2):
                    ps_sc = psA.tile([128, 2, 512], F32, tag="A", name="ps_sc")
                    for sub in range(2):
                        mc = 2 * mcp + sub
                        nc.tensor.matmul(
                            ps_sc[:, sub, :],
                            kv_sb[h][0:D, mc * 128:(mc + 1) * 128],
                            qt_sb[h][:, tqt * 512: tqt * 512 + 512],
                            start=True, stop=True)
                    nc.scalar.activation(
                        expT[:, 2 * mcp:2 * mcp + 2, :],
                        ps_sc[:],
                        mybir.ActivationFunctionType.Exp)
                ps_ov = psB.tile([128, 512], F32, tag="B", name="ps_ov")
                for mc in range(4):
                    nc.tensor.matmul(
                        ps_ov[0:D + 1, :],
                        vgp_sb[h][:, mc, :],
                        expT[:, mc, :],
                        start=(mc == 0), stop=(mc == 3))
                # rinv = 1/Z on DVE (fast approx), broadcast on GpSimd.
                # Z must reach SBUF partition 0 via a native op first: the
                # custom-DVE recip misreads partition-shifted PSUM on HW.
                zrow = rowp.tile([1, 512], F32, tag="zrow", name="zrow")
                nc.vector.tensor_copy(zrow[:], ps_ov[D:D + 1, :])
                rinv = rowp.tile([1, 512], F32, tag="rinv", name="rinv")
                nc.vector.reciprocal_approx_fast(rinv[:], zrow[:])
                rinv_b = rowp.tile([D, 512], F32, tag="rinv_b", name="rinv_b")
                nc.gpsimd.partition_broadcast(rinv_b[:], rinv[:])
                nc.vector.tensor_tensor(
                    outT_sb[h // 2][(h % 2) * D:(h % 2 + 1) * D,
                                    tqt * 512: tqt * 512 + 512],
                    ps_ov[0:D, :],
                    rinv_b[:],
                    op=mybir.AluOpType.mult)

            def po(t8):
                ps_po = psA.tile([128, 2, 512], F32, tag="A", name="ps_po")
                for eot in range(2):
                    for dd in range(2):
                        nc.tensor.matmul(
                            ps_po[:, eot, :],
                            outT_sb[dd][:, t8 * 128:(t8 + 1) * 128],
                            wo_sb[:, dd * E + eot * 512:
                                  dd * E + eot * 512 + 512],
                            start=(dd == 0), stop=(dd == 1))
                po_sb = pop.tile([128, 2, 512], BF16, tag="po", name="po_sb")
                if t8 % 2 == 0:
                    nc.scalar.copy(po_sb[:], ps_po[:])
                else:
                    nc.vector.tensor_copy(po_sb[:], ps_po[:])
                dma(out[t8 * 2], po_sb[:, 0, :])
                dma(out[t8 * 2 + 1], po_sb[:, 1, :])

            ph1(0, 0)
            ph1(1, 0)
            ph1(0, 1)
            ph1(1, 1)
            p2a(0)
            p2a(1)
            att(0, 0)
            p2a(2)
            att(0, 1)
            p2a(3)
            att(0, 2)
            att(0, 3)
            att(1, 0)
            po(0)
            po(1)
            att(1, 1)
            po(2)
            po(3)
            att(1, 2)
            att(1, 3)
            for t8 in range(4, 8):
                po(t8)

    nc.compile()
    return nc


_NC = None


def _get_nc():
    global _NC
    if _NC is None:
        _NC = build_program()
    return _NC


def shard_inputs(hidden_states, key_value_states, Wq, bq, Wk, bk, Wv, bv, Wo,
                 bo, stride):
    stride = int(stride)
    assert stride == STRIDE
    scale = float(D) ** -0.5
    bf = lambda a: np.ascontiguousarray(a).astype(NPBF16)
    in_maps = []
    for c in range(NCORES):
        b, g = divmod(c, 4)
        h0 = g * HPC
        r0, r1 = h0 * D, (h0 + HPC) * D
        # hsT [128, 8192]: col tqt*4096+e*512+t = hs[b][tqt*512+t, e*128+p]
        hsT_c = hidden_states[b].T.reshape(8, 128, 2, 512) \
            .transpose(1, 2, 0, 3).reshape(128, 8 * Tq)
        # wq [128, 2048]: col pair*1024+e*128+j = Wq_s[pair*128+j, e*128+p]
        Wqs = (Wq[r0:r1, :] * scale)
        wq_c = Wqs.T.reshape(8, 128, 2, 128).transpose(1, 2, 0, 3) \
            .reshape(128, 2048)
        # wkv [128, 4096]: col (h*8+e)*128+j: j<64 Wk, j>=64 Wv
        K3 = Wk[r0:r1, :].reshape(HPC, D, E)
        V3 = Wv[r0:r1, :].reshape(HPC, D, E)
        C = np.concatenate([K3, V3], axis=1)          # [4, 128, 1024]
        wkv_c = C.reshape(4, 128, 8, 128).transpose(3, 0, 2, 1) \
            .reshape(128, 4096)
        # kvg [4, 128, 4096]: col e*512+m = kv[b, h0+h+16m, e*128+p]
        kvg_c = np.empty((HPC, 128, 8 * M), NPBF16)
        for hl in range(HPC):
            R = key_value_states[b, (h0 + hl)::STRIDE, :]     # [512, 1024]
            kvg_c[hl] = bf(R.T.reshape(8, 128, M).transpose(1, 0, 2)
                           .reshape(128, 8 * M))
        # wo [128, 2048]: col dd*1024+n = Wo[n, r0+dd*128+p]
        wo_c = Wo[:, r0:r1].T.reshape(2, 128, E).transpose(1, 0, 2) \
            .reshape(128, 2048)
        bqp_c = (bq[r0:r1] * scale).astype(np.float32).reshape(2, 128, 1)
        in_maps.append({
            "hsT": bf(hsT_c),
            "wq": bf(wq_c),
            "wkv": bf(wkv_c),
            "kvg": np.ascontiguousarray(kvg_c),
            "wo": bf(wo_c),
            "bqp": bqp_c,
        })
    return in_maps


def combine_outputs(results, Wv, bv, Wo, bo):
    final_bias = (bv @ Wo.T + bo).astype(np.float32)  # [E]
    out = np.zeros((B, Tq, E), np.float32)
    for c in range(NCORES):
        b = c // 4
        # out dram [16, 128, 512]: chunk t8*2+eot
        o = results[c]["out"].astype(np.float32) \
            .reshape(8, 2, 128, 512).transpose(0, 2, 1, 3).reshape(Tq, E)
        out[b] += o
    out += final_bias[None, None, :]
    return out


def kernel(hidden_states, key_value_states, Wq, bq, Wk, bk, Wv, bv, Wo, bo,
           stride, _trace=False, _trace_kwargs=None):
    from concourse.bass_utils import run_bass_kernel_spmd

    args = [np.asarray(x, np.float32) for x in
            (hidden_states, key_value_states, Wq, bq, Wk, bk, Wv, bv, Wo, bo)]
    (hidden_states, key_value_states, Wq, bq, Wk, bk, Wv, bv, Wo, bo) = args
    in_maps = shard_inputs(hidden_states, key_value_states, Wq, bq, Wk, bk,
                           Wv, bv, Wo, bo, stride)
    nc = _get_nc()
    res = run_bass_kernel_spmd(
        nc, in_maps, list(range(NCORES)),
        trace=_trace, **(_trace_kwargs or {}))
    out = combine_outputs(res.results, Wv, bv, Wo, bo)
    kernel.last_run = res
    return out


# revision 22
# speedup vs baseline: 1.8913x; 1.0346x over previous
"""HEPOS BART cross-attention Trainium2 kernel (bf16, PE-dense rewrite).

Shapes (hardcoded): B=2, Tq=1024, Tk=8192, E=1024, H=16, D=64, stride=16,
m = Tk//stride = 512 keys per head.

Sharding: 8 cores = 2 batches x 4 head-groups (4 heads each).
All DMA'd tensors are bf16 (halves HBM traffic vs fp32; PE rate is the same
1 cycle/row as fp32r). PSUM accumulation is fp32 throughout.

Per core:
  phase 1: QT = (Wq_hg @ hs_b^T)*scale + bq  -> qt[h] [64, 1024] bf16
           (4 sequential psum groups of 8 matmuls; bias added on DVE)
  phase 2a per head: fused KV proj: stationary [Wk_h | Wv_h] chunk [128,128]
           -> psum [128, 512]: rows 0-63 KgT, 64-127 VgT. One DVE copy to
           SBUF; 4 PE transposes of the V half -> vgp [128, 4, 65]
           (col 64 = ones -> softmax denominator comes free in attn@V).
  phase 2b (tqt outer, h inner):
           scores: 4 matmuls into 2x [128, 2, 512] psum tiles
           exp: 2 scalar activations [128, 1024] -> expT bf16
           attn@V: 4 matmuls accum -> ps_ov [65, 512] (row 64 = Z)
           rinv = 1/Z: even heads on scalar as exp(-ln(Z)) (ln+exp share one
           ACT table -> no table thrash), odd heads on DVE reciprocal.
           PE broadcast ones x rinv -> ps_rb; DVE mult -> outT bf16.
  phase 3 (per tqt half): partial = outT^T @ WoT -> psum -> SBUF -> DMA.
bk is dropped (constant key shift cancels in softmax); bv folded into the
host-side final bias (bv @ Wo.T + bo).
"""

import numpy as np
import ml_dtypes

import concourse.bass as bass
import concourse.bacc as bacc
import concourse.tile as tile
from concourse import library_config, mybir
from concourse.masks import make_identity

B, Tq, Tk, E, H, D = 2, 1024, 8192, 1024, 16, 64
STRIDE = 16
M = Tk // STRIDE          # 512 keys per head
HPC = 4                   # heads per core
NCORES = 8
F32 = mybir.dt.float32
BF16 = mybir.dt.bfloat16
NPBF16 = ml_dtypes.bfloat16


def build_program():
    nc = bacc.Bacc("TRN2", target_bir_lowering=False)

    # dram tensors already in SBUF layout
    # hsT col = tqt*4096 + e*512 + t ; wq col = pair*1024 + e*128 + j
    hsT = nc.dram_tensor("hsT", [128, 8 * Tq], BF16, kind="ExternalInput")
    wq = nc.dram_tensor("wq", [128, 2048], BF16, kind="ExternalInput")
    wkv = nc.dram_tensor("wkv", [128, 4096], BF16, kind="ExternalInput")
    kvg = nc.dram_tensor("kvg", [HPC, 128, 8 * M], BF16, kind="ExternalInput")
    wo = nc.dram_tensor("wo", [128, 2048], BF16, kind="ExternalInput")
    bqp = nc.dram_tensor("bqp", [2, 128, 1], F32, kind="ExternalInput")
    out = nc.dram_tensor("out", [16, 128, 512], BF16, kind="ExternalOutput")

    _dma_engs = [None, None]
    _dma_i = [0]

    def dma(out_ap, in_ap):
        eng = _dma_engs[_dma_i[0] % 2]
        _dma_i[0] += 1
        eng.dma_start(out=out_ap, in_=in_ap)

    with tile.TileContext(nc) as tc:
        _dma_engs[0] = nc.sync
        _dma_engs[1] = nc.gpsimd
        with (
            tc.tile_pool(name="consts", bufs=1) as consts,
            tc.tile_pool(name="expp", bufs=2) as expp,
            tc.tile_pool(name="rowp", bufs=2) as rowp,
            tc.tile_pool(name="pop", bufs=3) as pop,
            tc.tile_pool(name="psA", bufs=2, space="PSUM") as psA,
            tc.tile_pool(name="psB", bufs=4, space="PSUM") as psB,
        ):
            # ---- persistent SBUF tiles -------------------------------------
            hsT_sb = consts.tile([128, 8 * Tq], BF16)
            wq_sb = consts.tile([128, 2048], BF16)
            wkv_sb = consts.tile([128, 4096], BF16)
            wo_sb = consts.tile([128, 2048], BF16)
            kvg_sb = [consts.tile([128, 8 * M], BF16, name=f"kvg{h}")
                      for h in range(HPC)]
            kv_sb = [consts.tile([128, M], BF16, name=f"kv{h}")
                     for h in range(HPC)]
            vgp_sb = [consts.tile([128, 4, D + 1], BF16, name=f"vgp{h}")
                      for h in range(HPC)]
            qt_sb = [consts.tile([D, Tq], BF16, name=f"qt{h}")
                     for h in range(HPC)]
            outT_sb = [consts.tile([128, Tq], BF16, name=f"outT{dd}")
                       for dd in range(2)]
            bq_sb = [consts.tile([128, 1], F32, name=f"bq{p}") for p in range(2)]

            identf = consts.tile([128, 128], F32)
            make_identity(nc, identf)
            ident = consts.tile([128, 128], BF16)
            nc.vector.tensor_copy(ident[:], identf[:])
            for h in range(HPC):
                nc.vector.memset(vgp_sb[h][:, :, D:D + 1], 1.0)

            # ---- input DMAs, two waves ------------------------------------
            # Wave 1 (phase1 + head0) gets the queues to itself so its bytes
            # land early; later waves are gated behind it via tiny gpsimd
            # reads (DMA queues round-robin between all enqueued transfers,
            # so an ungated bulk enqueue starves the critical path).
            dma(wq_sb[:, 0:1024], wq[:, 0:1024])                   # sync
            dma(hsT_sb[:, 0:2048], hsT[:, 0:2048])                 # gpsimd
            dma(hsT_sb[:, 2048:4096], hsT[:, 2048:4096])           # sync
            dma(wq_sb[:, 1024:2048], wq[:, 1024:2048])             # gpsimd
            dma(kvg_sb[0][:, 0:2048], kvg[0][:, 0:2048])           # sync
            dma(kvg_sb[0][:, 2048:4096], kvg[0][:, 2048:4096])     # gpsimd
            dma(hsT_sb[:, 4096:6144], hsT[:, 4096:6144])           # sync
            dma(hsT_sb[:, 6144:8192], hsT[:, 6144:8192])           # gpsimd
            dma(wkv_sb[:, 0:1024], wkv[:, 0:1024])                 # sync
            dma(bq_sb[0][:], bqp[0])
            dma(bq_sb[1][:], bqp[1])

            gate_sb = consts.tile([1, 8], BF16)
            _gate_i = [0]

            def gate(sl):
                # 4-byte SBUF->SBUF DMA on sync whose read-dep stalls sync
                # (and thus all later sync dispatches) until `sl`'s writer
                # DMA has landed.
                i = _gate_i[0]
                _gate_i[0] += 1
                nc.sync.dma_start(out=gate_sb[:, i:i + 1], in_=sl)

            gate(kvg_sb[0][127:128, 4095:4096])
            gate(hsT_sb[127:128, 8191:8192])
            nc.sync.dma_start(out=kvg_sb[1][:, 0:2048],
                              in_=kvg[1][:, 0:2048])
            nc.sync.dma_start(out=kvg_sb[1][:, 2048:4096],
                              in_=kvg[1][:, 2048:4096])
            nc.sync.dma_start(out=wkv_sb[:, 1024:2048],
                              in_=wkv[:, 1024:2048])
            gate(kvg_sb[1][127:128, 4095:4096])
            nc.sync.dma_start(out=kvg_sb[2][:, 0:2048],
                              in_=kvg[2][:, 0:2048])
            nc.sync.dma_start(out=kvg_sb[2][:, 2048:4096],
                              in_=kvg[2][:, 2048:4096])
            nc.sync.dma_start(out=wkv_sb[:, 2048:3072],
                              in_=wkv[:, 2048:3072])
            gate(kvg_sb[2][127:128, 4095:4096])
            nc.sync.dma_start(out=kvg_sb[3][:, 0:2048],
                              in_=kvg[3][:, 0:2048])
            nc.sync.dma_start(out=kvg_sb[3][:, 2048:4096],
                              in_=kvg[3][:, 2048:4096])
            nc.sync.dma_start(out=wkv_sb[:, 3072:4096],
                              in_=wkv[:, 3072:4096])
            gate(kvg_sb[3][127:128, 4095:4096])
            nc.sync.dma_start(out=wo_sb[:, 0:1024], in_=wo[:, 0:1024])
            nc.sync.dma_start(out=wo_sb[:, 1024:2048],
                              in_=wo[:, 1024:2048])

            # ---- stage closures (issued in an interleaved order so the
            # in-order PE queue never head-of-line blocks on a late DMA) ----
            def ph1(pair, tqt):
                ps_qt = psA.tile([128, 2, 512], F32, tag="A", name="ps_qt")
                for e in range(8):
                    nc.tensor.matmul(
                        ps_qt[:, 0, :],
                        wq_sb[:, pair * 1024 + e * 128:
                              pair * 1024 + (e + 1) * 128],
                        hsT_sb[:, tqt * 4096 + e * 512:
                               tqt * 4096 + (e + 1) * 512],
                        start=(e == 0), stop=(e == 7))
                for sub in range(2):
                    h = 2 * pair + sub
                    nc.vector.tensor_scalar_add(
                        qt_sb[h][:, tqt * 512: tqt * 512 + 512],
                        ps_qt[sub * 64:(sub + 1) * 64, 0, :],
                        bq_sb[pair][sub * 64:(sub + 1) * 64, 0:1])

            def p2a(h):
                ps_kv = psA.tile([128, 2, 512], F32, tag="A", name="ps_kv")
                for e in range(8):
                    nc.tensor.matmul(
                        ps_kv[:, 0, :],
                        wkv_sb[:, (h * 8 + e) * 128:(h * 8 + e + 1) * 128],
                        kvg_sb[h][:, e * M:(e + 1) * M],
                        start=(e == 0), stop=(e == 7))
                nc.scalar.copy(kv_sb[h][:], ps_kv[:, 0, :])
                ps_vt = psB.tile([128, 4, D], BF16, tag="B", name="ps_vt")
                for mc in range(4):
                    nc.tensor.transpose(
                        ps_vt[:, mc, :],
                        kv_sb[h][64:128, mc * 128:(mc + 1) * 128],
                        ident[64:128, 64:128])
                nc.vector.tensor_copy(vgp_sb[h][:, :, 0:D], ps_vt[:])

            pending = []

            def flush():
                while pending:
                    pending.pop(0)()

            def att(tqt, h):
                if pending:
                    pending.pop(0)()
                expT = expp.tile([128, 4, 512], BF16, tag="expT", name="expT")
                for mcp in range(2):
                    ps_sc = psA.tile([128, 2, 512], F32, tag="A", name="ps_sc")
                    for sub in range(2):
                        mc = 2 * mcp + sub
                        nc.tensor.matmul(
                            ps_sc[:, sub, :],
                            kv_sb[h][0:D, mc * 128:(mc + 1) * 128],
                            qt_sb[h][:, tqt * 512: tqt * 512 + 512],
                            start=True, stop=True)
                    nc.scalar.activation(
                        expT[:, 2 * mcp:2 * mcp + 2, :],
                        ps_sc[:],
                        mybir.ActivationFunctionType.Exp)
                ps_ov = psB.tile([128, 512], F32, tag="B", name="ps_ov")
                for mc in range(4):
                    nc.tensor.matmul(
                        ps_ov[0:D + 1, :],
                        vgp_sb[h][:, mc, :],
                        expT[:, mc, :],
                        start=(mc == 0), stop=(mc == 3))
                # rinv = 1/Z on DVE (fast approx), broadcast on GpSimd.
                # Z must reach SBUF partition 0 via a native op first: the
                # custom-DVE recip misreads partition-shifted PSUM on HW.
                zrow = rowp.tile([1, 512], F32, tag="zrow", name="zrow")
                nc.vector.tensor_copy(zrow[:], ps_ov[D:D + 1, :])
                rinv = rowp.tile([1, 512], F32, tag="rinv", name="rinv")
                nc.vector.reciprocal_approx_fast(rinv[:], zrow[:])
                rinv_b = rowp.tile([D, 512], F32, tag="rinv_b", name="rinv_b")
                nc.gpsimd.partition_broadcast(rinv_b[:], rinv[:])

                # defer the normalize-mult by one stage so the vector queue
                # doesn't stall waiting on the gpsimd broadcast
                def _mult(tqt=tqt, h=h, ps_ov=ps_ov, rinv_b=rinv_b):
                    nc.vector.tensor_tensor(
                        outT_sb[h // 2][(h % 2) * D:(h % 2 + 1) * D,
                                        tqt * 512: tqt * 512 + 512],
                        ps_ov[0:D, :],
                        rinv_b[:],
                        op=mybir.AluOpType.mult)
                pending.append(_mult)

            def po(t8):
                flush()
                ps_po = psA.tile([128, 2, 512], F32, tag="A", name="ps_po")
                for eot in range(2):
                    for dd in range(2):
                        nc.tensor.matmul(
                            ps_po[:, eot, :],
                            outT_sb[dd][:, t8 * 128:(t8 + 1) * 128],
                            wo_sb[:, dd * E + eot * 512:
                                  dd * E + eot * 512 + 512],
                            start=(dd == 0), stop=(dd == 1))
                po_sb = pop.tile([128, 2, 512], BF16, tag="po", name="po_sb")
                if t8 % 2 == 0:
                    nc.scalar.copy(po_sb[:], ps_po[:])
                else:
                    nc.vector.tensor_copy(po_sb[:], ps_po[:])
                dma(out[t8 * 2], po_sb[:, 0, :])
                dma(out[t8 * 2 + 1], po_sb[:, 1, :])

            ph1(0, 0)
            ph1(1, 0)
            ph1(0, 1)
            ph1(1, 1)
            p2a(0)
            p2a(1)
            att(0, 0)
            p2a(2)
            att(0, 1)
            p2a(3)
            att(0, 2)
            att(0, 3)
            att(1, 0)
            po(0)
            po(1)
            att(1, 1)
            po(2)
            po(3)
            att(1, 2)
            att(1, 3)
            for t8 in range(4, 8):
                po(t8)

    nc.compile()
    return nc


_NC = None


def _get_nc():
    global _NC
    if _NC is None:
        _NC = build_program()
    return _NC


def shard_inputs(hidden_states, key_value_states, Wq, bq, Wk, bk, Wv, bv, Wo,
                 bo, stride):
    stride = int(stride)
    assert stride == STRIDE
    scale = float(D) ** -0.5
    bf = lambda a: np.ascontiguousarray(a).astype(NPBF16)
    in_maps = []
    for c in range(NCORES):
        b, g = divmod(c, 4)
        h0 = g * HPC
        r0, r1 = h0 * D, (h0 + HPC) * D
        # hsT [128, 8192]: col tqt*4096+e*512+t = hs[b][tqt*512+t, e*128+p]
        hsT_c = hidden_states[b].T.reshape(8, 128, 2, 512) \
            .transpose(1, 2, 0, 3).reshape(128, 8 * Tq)
        # wq [128, 2048]: col pair*1024+e*128+j = Wq_s[pair*128+j, e*128+p]
        Wqs = (Wq[r0:r1, :] * scale)
        wq_c = Wqs.T.reshape(8, 128, 2, 128).transpose(1, 2, 0, 3) \
            .reshape(128, 2048)
        # wkv [128, 4096]: col (h*8+e)*128+j: j<64 Wk, j>=64 Wv
        K3 = Wk[r0:r1, :].reshape(HPC, D, E)
        V3 = Wv[r0:r1, :].reshape(HPC, D, E)
        C = np.concatenate([K3, V3], axis=1)          # [4, 128, 1024]
        wkv_c = C.reshape(4, 128, 8, 128).transpose(3, 0, 2, 1) \
            .reshape(128, 4096)
        # kvg [4, 128, 4096]: col e*512+m = kv[b, h0+h+16m, e*128+p]
        kvg_c = np.empty((HPC, 128, 8 * M), NPBF16)
        for hl in range(HPC):
            R = key_value_states[b, (h0 + hl)::STRIDE, :]     # [512, 1024]
            kvg_c[hl] = bf(R.T.reshape(8, 128, M).transpose(1, 0, 2)
                           .reshape(128, 8 * M))
        # wo [128, 2048]: col dd*1024+n = Wo[n, r0+dd*128+p]
        wo_c = Wo[:, r0:r1].T.reshape(2, 128, E).transpose(1, 0, 2) \
            .reshape(128, 2048)
        bqp_c = (bq[r0:r1] * scale).astype(np.float32).reshape(2, 128, 1)
        in_maps.append({
            "hsT": bf(hsT_c),
            "wq": bf(wq_c),
            "wkv": bf(wkv_c),
            "kvg": np.ascontiguousarray(kvg_c),
            "wo": bf(wo_c),
            "bqp": bqp_c,
        })
    return in_maps


def combine_outputs(results, Wv, bv, Wo, bo):
    final_bias = (bv @ Wo.T + bo).astype(np.float32)  # [E]
    out = np.zeros((B, Tq, E), np.float32)
    for c in range(NCORES):
        b = c // 4
        # out dram [16, 128, 512]: chunk t8*2+eot
        o = results[c]["out"].astype(np.float32) \
            .reshape(8, 2, 128, 512).transpose(0, 2, 1, 3).reshape(Tq, E)
        out[b] += o
    out += final_bias[None, None, :]
    return out


def kernel(hidden_states, key_value_states, Wq, bq, Wk, bk, Wv, bv, Wo, bo,
           stride, _trace=False, _trace_kwargs=None):
    from concourse.bass_utils import run_bass_kernel_spmd

    args = [np.asarray(x, np.float32) for x in
            (hidden_states, key_value_states, Wq, bq, Wk, bk, Wv, bv, Wo, bo)]
    (hidden_states, key_value_states, Wq, bq, Wk, bk, Wv, bv, Wo, bo) = args
    in_maps = shard_inputs(hidden_states, key_value_states, Wq, bq, Wk, bk,
                           Wv, bv, Wo, bo, stride)
    nc = _get_nc()
    res = run_bass_kernel_spmd(
        nc, in_maps, list(range(NCORES)),
        trace=_trace, **(_trace_kwargs or {}))
    out = combine_outputs(res.results, Wv, bv, Wo, bo)
    kernel.last_run = res
    return out


# revision 44
# speedup vs baseline: 1.9856x; 1.0499x over previous
"""HEPOS BART cross-attention Trainium2 kernel (bf16, PE-dense rewrite).

Shapes (hardcoded): B=2, Tq=1024, Tk=8192, E=1024, H=16, D=64, stride=16,
m = Tk//stride = 512 keys per head.

Sharding: 8 cores = 2 batches x 4 head-groups (4 heads each).
All DMA'd tensors are bf16 (halves HBM traffic vs fp32; PE rate is the same
1 cycle/row as fp32r). PSUM accumulation is fp32 throughout.

Per core:
  phase 1: QT = (Wq_hg @ hs_b^T)*scale + bq  -> qt[h] [64, 1024] bf16
           (4 sequential psum groups of 8 matmuls; bias added on DVE)
  phase 2a per head: fused KV proj: stationary [Wk_h | Wv_h] chunk [128,128]
           -> psum [128, 512]: rows 0-63 KgT, 64-127 VgT. One DVE copy to
           SBUF; 4 PE transposes of the V half -> vgp [128, 4, 65]
           (col 64 = ones -> softmax denominator comes free in attn@V).
  phase 2b (tqt outer, h inner):
           scores: 4 matmuls into 2x [128, 2, 512] psum tiles
           exp: 2 scalar activations [128, 1024] -> expT bf16
           attn@V: 4 matmuls accum -> ps_ov [65, 512] (row 64 = Z)
           rinv = 1/Z: even heads on scalar as exp(-ln(Z)) (ln+exp share one
           ACT table -> no table thrash), odd heads on DVE reciprocal.
           PE broadcast ones x rinv -> ps_rb; DVE mult -> outT bf16.
  phase 3 (per tqt half): partial = outT^T @ WoT -> psum -> SBUF -> DMA.
bk is dropped (constant key shift cancels in softmax); bv folded into the
host-side final bias (bv @ Wo.T + bo).
"""

import numpy as np
import ml_dtypes

import concourse.bass as bass
import concourse.bacc as bacc
import concourse.tile as tile
from concourse import library_config, mybir
from concourse.masks import make_identity

B, Tq, Tk, E, H, D = 2, 1024, 8192, 1024, 16, 64
STRIDE = 16
M = Tk // STRIDE          # 512 keys per head
HPC = 4                   # heads per core
NCORES = 8
F32 = mybir.dt.float32
BF16 = mybir.dt.bfloat16
NPBF16 = ml_dtypes.bfloat16


def build_program():
    nc = bacc.Bacc("TRN2", target_bir_lowering=False)

    # dram tensors already in SBUF layout
    # hsT col = tqt*4096 + e*512 + t ; wq col = pair*1024 + e*128 + j
    hsT = nc.dram_tensor("hsT", [128, 8 * Tq], BF16, kind="ExternalInput")
    wq = nc.dram_tensor("wq", [128, 2048], BF16, kind="ExternalInput")
    wkv = nc.dram_tensor("wkv", [128, 4096], BF16, kind="ExternalInput")
    kvg = nc.dram_tensor("kvg", [HPC, 128, 8 * M], BF16, kind="ExternalInput")
    wo = nc.dram_tensor("wo", [128, 2048], BF16, kind="ExternalInput")
    bqp = nc.dram_tensor("bqp", [2, 128, 1], F32, kind="ExternalInput")
    out = nc.dram_tensor("out", [16, 128, 512], BF16, kind="ExternalOutput")

    _dma_engs = [None, None]
    _dma_i = [0]

    def dma(out_ap, in_ap):
        eng = _dma_engs[_dma_i[0] % 2]
        _dma_i[0] += 1
        eng.dma_start(out=out_ap, in_=in_ap)

    with tile.TileContext(nc) as tc:
        _dma_engs[0] = nc.sync
        _dma_engs[1] = nc.gpsimd
        with (
            tc.tile_pool(name="consts", bufs=1) as consts,
            tc.tile_pool(name="expp", bufs=3) as expp,
            tc.tile_pool(name="rowp", bufs=3) as rowp,
            tc.tile_pool(name="pop", bufs=3) as pop,
            tc.tile_pool(name="psA", bufs=3, space="PSUM") as psA,
            tc.tile_pool(name="psB", bufs=2, space="PSUM") as psB,
        ):
            # ---- persistent SBUF tiles -------------------------------------
            hsT_sb = consts.tile([128, 8 * Tq], BF16)
            wq_sb = consts.tile([128, 2048], BF16)
            wkv_sb = consts.tile([128, 4096], BF16)
            wo_sb = consts.tile([128, 2048], BF16)
            kvg_sb = [consts.tile([128, 8 * M], BF16, name=f"kvg{h}")
                      for h in range(HPC)]
            kv_sb = [consts.tile([128, M], BF16, name=f"kv{h}")
                     for h in range(HPC)]
            vgp_sb = [consts.tile([128, 4, D + 1], BF16, name=f"vgp{h}")
                      for h in range(HPC)]
            qt_sb = [consts.tile([D, Tq], BF16, name=f"qt{h}")
                     for h in range(HPC)]
            outT_sb = [consts.tile([128, Tq], BF16, name=f"outT{dd}")
                       for dd in range(2)]
            bq_sb = [consts.tile([128, 1], F32, name=f"bq{p}") for p in range(2)]

            identf = consts.tile([128, 128], F32)
            make_identity(nc, identf)
            ident = consts.tile([128, 128], BF16)
            nc.vector.tensor_copy(ident[:], identf[:])
            for h in range(HPC):
                nc.vector.memset(vgp_sb[h][:, :, D:D + 1], 1.0)

            # ---- input DMAs, two waves ------------------------------------
            # Wave 1 (phase1 + head0) gets the queues to itself so its bytes
            # land early; later waves are gated behind it via tiny gpsimd
            # reads (DMA queues round-robin between all enqueued transfers,
            # so an ungated bulk enqueue starves the critical path).
            gate_sb = consts.tile([1, 8], BF16)
            _gate_i = [0]

            def gate(sl):
                # 4-byte SBUF->SBUF DMA on sync whose read-dep stalls sync
                # (and thus all later sync dispatches) until `sl`'s writer
                # DMA has landed.
                i = _gate_i[0]
                _gate_i[0] += 1
                nc.sync.dma_start(out=gate_sb[:, i:i + 1], in_=sl)

            # wave 1 (ungated): phase-1 data + head-0 K/V
            dma(wq_sb[:, 0:1024], wq[:, 0:1024])                   # sync
            dma(hsT_sb[:, 0:2048], hsT[:, 0:2048])                 # gpsimd
            dma(hsT_sb[:, 2048:4096], hsT[:, 2048:4096])           # sync
            dma(wq_sb[:, 1024:2048], wq[:, 1024:2048])             # gpsimd
            dma(kvg_sb[0][:, 0:2048], kvg[0][:, 0:2048])           # sync
            dma(kvg_sb[0][:, 2048:4096], kvg[0][:, 2048:4096])     # gpsimd
            dma(hsT_sb[:, 4096:6144], hsT[:, 4096:6144])           # sync
            dma(hsT_sb[:, 6144:8192], hsT[:, 6144:8192])           # gpsimd
            dma(wkv_sb[:, 0:1024], wkv[:, 0:1024])                 # sync
            dma(bq_sb[0][:], bqp[0])
            dma(bq_sb[1][:], bqp[1])
            # waves 2..5 gated per head so each head's bytes land in order
            gate(kvg_sb[0][127:128, 4095:4096])
            gate(hsT_sb[127:128, 8191:8192])
            nc.sync.dma_start(out=kvg_sb[1][:, 0:2048],
                              in_=kvg[1][:, 0:2048])
            nc.sync.dma_start(out=kvg_sb[1][:, 2048:4096],
                              in_=kvg[1][:, 2048:4096])
            nc.sync.dma_start(out=wkv_sb[:, 1024:2048],
                              in_=wkv[:, 1024:2048])
            gate(kvg_sb[1][127:128, 4095:4096])
            nc.sync.dma_start(out=kvg_sb[2][:, 0:2048],
                              in_=kvg[2][:, 0:2048])
            nc.sync.dma_start(out=kvg_sb[2][:, 2048:4096],
                              in_=kvg[2][:, 2048:4096])
            nc.sync.dma_start(out=wkv_sb[:, 2048:3072],
                              in_=wkv[:, 2048:3072])
            gate(kvg_sb[2][127:128, 4095:4096])
            nc.sync.dma_start(out=kvg_sb[3][:, 0:2048],
                              in_=kvg[3][:, 0:2048])
            nc.sync.dma_start(out=kvg_sb[3][:, 2048:4096],
                              in_=kvg[3][:, 2048:4096])
            nc.sync.dma_start(out=wkv_sb[:, 3072:4096],
                              in_=wkv[:, 3072:4096])
            gate(kvg_sb[3][127:128, 4095:4096])
            nc.sync.dma_start(out=wo_sb[:, 0:1024], in_=wo[:, 0:1024])
            nc.sync.dma_start(out=wo_sb[:, 1024:2048],
                              in_=wo[:, 1024:2048])

            # ---- stage closures (issued in an interleaved order so the
            # in-order PE queue never head-of-line blocks on a late DMA) ----
            def ph1(pair, tqt):
                ps_qt = psA.tile([128, 2, 512], F32, tag="A", name="ps_qt")
                for e in range(8):
                    nc.tensor.matmul(
                        ps_qt[:, 0, :],
                        wq_sb[:, pair * 1024 + e * 128:
                              pair * 1024 + (e + 1) * 128],
                        hsT_sb[:, tqt * 4096 + e * 512:
                               tqt * 4096 + (e + 1) * 512],
                        start=(e == 0), stop=(e == 7))
                for sub in range(2):
                    h = 2 * pair + sub
                    nc.vector.tensor_scalar_add(
                        qt_sb[h][:, tqt * 512: tqt * 512 + 512],
                        ps_qt[sub * 64:(sub + 1) * 64, 0, :],
                        bq_sb[pair][sub * 64:(sub + 1) * 64, 0:1])

            def p2a(h):
                ps_kv = psA.tile([128, 2, 512], F32, tag="A", name="ps_kv")
                for e in range(8):
                    nc.tensor.matmul(
                        ps_kv[:, 0, :],
                        wkv_sb[:, (h * 8 + e) * 128:(h * 8 + e + 1) * 128],
                        kvg_sb[h][:, e * M:(e + 1) * M],
                        start=(e == 0), stop=(e == 7))
                nc.scalar.copy(kv_sb[h][:], ps_kv[:, 0, :])
                ps_vt = psB.tile([128, 4, D], BF16, tag="B", name="ps_vt")
                for mc in range(4):
                    nc.tensor.transpose(
                        ps_vt[:, mc, :],
                        kv_sb[h][64:128, mc * 128:(mc + 1) * 128],
                        ident[64:128, 64:128])
                nc.vector.tensor_copy(vgp_sb[h][:, :, 0:D], ps_vt[:])

            pending = []

            def flush():
                while pending:
                    pending.pop(0)()

            def sc_part(tqt, h):
                expT = expp.tile([128, 4, 512], BF16, tag="expT", name="expT")
                for mcp in range(2):
                    ps_sc = psA.tile([128, 2, 512], F32, tag="A", name="ps_sc")
                    for sub in range(2):
                        mc = 2 * mcp + sub
                        nc.tensor.matmul(
                            ps_sc[:, sub, :],
                            kv_sb[h][0:D, mc * 128:(mc + 1) * 128],
                            qt_sb[h][:, tqt * 512: tqt * 512 + 512],
                            start=True, stop=True)
                    nc.scalar.activation(
                        expT[:, 2 * mcp:2 * mcp + 2, :],
                        ps_sc[:],
                        mybir.ActivationFunctionType.Exp)
                return expT

            def av_part(tqt, h, expT):
                if pending:
                    pending.pop(0)()
                ps_ov = psB.tile([128, 512], F32, tag="B", name="ps_ov")
                for mc in range(4):
                    nc.tensor.matmul(
                        ps_ov[0:D + 1, :],
                        vgp_sb[h][:, mc, :],
                        expT[:, mc, :],
                        start=(mc == 0), stop=(mc == 3))
                # rinv = 1/Z on DVE (fast approx), broadcast on GpSimd.
                # Z must reach SBUF partition 0 via a native op first: the
                # custom-DVE recip misreads partition-shifted PSUM on HW.
                zrow = rowp.tile([1, 512], F32, tag="zrow", name="zrow")
                nc.vector.tensor_copy(zrow[:], ps_ov[D:D + 1, :])
                rinv = rowp.tile([1, 512], F32, tag="rinv", name="rinv")
                nc.vector.reciprocal_approx_fast(rinv[:], zrow[:])
                rinv_b = rowp.tile([D, 512], F32, tag="rinv_b", name="rinv_b")
                nc.gpsimd.partition_broadcast(rinv_b[:], rinv[:])

                # defer the normalize-mult by one stage so the vector queue
                # doesn't stall waiting on the gpsimd broadcast
                def _mult(tqt=tqt, h=h, ps_ov=ps_ov, rinv_b=rinv_b):
                    nc.vector.tensor_tensor(
                        outT_sb[h // 2][(h % 2) * D:(h % 2 + 1) * D,
                                        tqt * 512: tqt * 512 + 512],
                        ps_ov[0:D, :],
                        rinv_b[:],
                        op=mybir.AluOpType.mult)
                pending.append(_mult)

            def po(t8):
                flush()
                ps_po = psA.tile([128, 2, 512], F32, tag="A", name="ps_po")
                for eot in range(2):
                    for dd in range(2):
                        nc.tensor.matmul(
                            ps_po[:, eot, :],
                            outT_sb[dd][:, t8 * 128:(t8 + 1) * 128],
                            wo_sb[:, dd * E + eot * 512:
                                  dd * E + eot * 512 + 512],
                            start=(dd == 0), stop=(dd == 1))
                po_sb = pop.tile([128, 2, 512], BF16, tag="po", name="po_sb")
                nc.scalar.copy(po_sb[:, 0, :], ps_po[:, 0, :])
                nc.vector.tensor_copy(po_sb[:, 1, :], ps_po[:, 1, :])
                dma(out[t8 * 2], po_sb[:, 0, :])
                dma(out[t8 * 2 + 1], po_sb[:, 1, :])

            ph1(0, 0)
            ph1(1, 0)
            ph1(0, 1)
            ph1(1, 1)
            p2a(0)
            p2a(1)
            e00 = sc_part(0, 0)
            av_part(0, 0, e00)
            e01 = sc_part(0, 1)
            av_part(0, 1, e01)
            p2a(2)
            e02 = sc_part(0, 2)
            av_part(0, 2, e02)
            p2a(3)
            e03 = sc_part(0, 3)
            av_part(0, 3, e03)
            e10 = sc_part(1, 0)
            av_part(1, 0, e10)
            po(0)
            po(1)
            e11 = sc_part(1, 1)
            av_part(1, 1, e11)
            po(2)
            po(3)
            e12 = sc_part(1, 2)
            av_part(1, 2, e12)
            e13 = sc_part(1, 3)
            av_part(1, 3, e13)
            for t8 in range(4, 8):
                po(t8)

    nc.compile()
    return nc


_NC = None


def _get_nc():
    global _NC
    if _NC is None:
        _NC = build_program()
    return _NC


def shard_inputs(hidden_states, key_value_states, Wq, bq, Wk, bk, Wv, bv, Wo,
                 bo, stride):
    stride = int(stride)
    assert stride == STRIDE
    scale = float(D) ** -0.5
    bf = lambda a: np.ascontiguousarray(a).astype(NPBF16)
    in_maps = []
    for c in range(NCORES):
        b, g = divmod(c, 4)
        h0 = g * HPC
        r0, r1 = h0 * D, (h0 + HPC) * D
        # hsT [128, 8192]: col tqt*4096+e*512+t = hs[b][tqt*512+t, e*128+p]
        hsT_c = hidden_states[b].T.reshape(8, 128, 2, 512) \
            .transpose(1, 2, 0, 3).reshape(128, 8 * Tq)
        # wq [128, 2048]: col pair*1024+e*128+j = Wq_s[pair*128+j, e*128+p]
        Wqs = (Wq[r0:r1, :] * scale)
        wq_c = Wqs.T.reshape(8, 128, 2, 128).transpose(1, 2, 0, 3) \
            .reshape(128, 2048)
        # wkv [128, 4096]: col (h*8+e)*128+j: j<64 Wk, j>=64 Wv
        K3 = Wk[r0:r1, :].reshape(HPC, D, E)
        V3 = Wv[r0:r1, :].reshape(HPC, D, E)
        C = np.concatenate([K3, V3], axis=1)          # [4, 128, 1024]
        wkv_c = C.reshape(4, 128, 8, 128).transpose(3, 0, 2, 1) \
            .reshape(128, 4096)
        # kvg [4, 128, 4096]: col e*512+m = kv[b, h0+h+16m, e*128+p]
        kvg_c = np.empty((HPC, 128, 8 * M), NPBF16)
        for hl in range(HPC):
            R = key_value_states[b, (h0 + hl)::STRIDE, :]     # [512, 1024]
            kvg_c[hl] = bf(R.T.reshape(8, 128, M).transpose(1, 0, 2)
                           .reshape(128, 8 * M))
        # wo [128, 2048]: col dd*1024+n = Wo[n, r0+dd*128+p]
        wo_c = Wo[:, r0:r1].T.reshape(2, 128, E).transpose(1, 0, 2) \
            .reshape(128, 2048)
        bqp_c = (bq[r0:r1] * scale).astype(np.float32).reshape(2, 128, 1)
        in_maps.append({
            "hsT": bf(hsT_c),
            "wq": bf(wq_c),
            "wkv": bf(wkv_c),
            "kvg": np.ascontiguousarray(kvg_c),
            "wo": bf(wo_c),
            "bqp": bqp_c,
        })
    return in_maps


def combine_outputs(results, Wv, bv, Wo, bo):
    final_bias = (bv @ Wo.T + bo).astype(np.float32)  # [E]
    out = np.zeros((B, Tq, E), np.float32)
    for c in range(NCORES):
        b = c // 4
        # out dram [16, 128, 512]: chunk t8*2+eot
        o = results[c]["out"].astype(np.float32) \
            .reshape(8, 2, 128, 512).transpose(0, 2, 1, 3).reshape(Tq, E)
        out[b] += o
    out += final_bias[None, None, :]
    return out


def kernel(hidden_states, key_value_states, Wq, bq, Wk, bk, Wv, bv, Wo, bo,
           stride, _trace=False, _trace_kwargs=None):
    from concourse.bass_utils import run_bass_kernel_spmd

    args = [np.asarray(x, np.float32) for x in
            (hidden_states, key_value_states, Wq, bq, Wk, bk, Wv, bv, Wo, bo)]
    (hidden_states, key_value_states, Wq, bq, Wk, bk, Wv, bv, Wo, bo) = args
    in_maps = shard_inputs(hidden_states, key_value_states, Wq, bq, Wk, bk,
                           Wv, bv, Wo, bo, stride)
    nc = _get_nc()
    res = run_bass_kernel_spmd(
        nc, in_maps, list(range(NCORES)),
        trace=_trace, **(_trace_kwargs or {}))
    out = combine_outputs(res.results, Wv, bv, Wo, bo)
    kernel.last_run = res
    return out


# revision 45
# speedup vs baseline: 1.9891x; 1.0017x over previous
"""HEPOS BART cross-attention Trainium2 kernel (bf16, PE-dense rewrite).

Shapes (hardcoded): B=2, Tq=1024, Tk=8192, E=1024, H=16, D=64, stride=16,
m = Tk//stride = 512 keys per head.

Sharding: 8 cores = 2 batches x 4 head-groups (4 heads each).
All DMA'd tensors are bf16 (halves HBM traffic vs fp32; PE rate is the same
1 cycle/row as fp32r). PSUM accumulation is fp32 throughout.

Per core:
  phase 1: QT = (Wq_hg @ hs_b^T)*scale + bq  -> qt[h] [64, 1024] bf16
           (4 sequential psum groups of 8 matmuls; bias added on DVE)
  phase 2a per head: fused KV proj: stationary [Wk_h | Wv_h] chunk [128,128]
           -> psum [128, 512]: rows 0-63 KgT, 64-127 VgT. One DVE copy to
           SBUF; 4 PE transposes of the V half -> vgp [128, 4, 65]
           (col 64 = ones -> softmax denominator comes free in attn@V).
  phase 2b (tqt outer, h inner):
           scores: 4 matmuls into 2x [128, 2, 512] psum tiles
           exp: 2 scalar activations [128, 1024] -> expT bf16
           attn@V: 4 matmuls accum -> ps_ov [65, 512] (row 64 = Z)
           rinv = 1/Z: even heads on scalar as exp(-ln(Z)) (ln+exp share one
           ACT table -> no table thrash), odd heads on DVE reciprocal.
           PE broadcast ones x rinv -> ps_rb; DVE mult -> outT bf16.
  phase 3 (per tqt half): partial = outT^T @ WoT -> psum -> SBUF -> DMA.
bk is dropped (constant key shift cancels in softmax); bv folded into the
host-side final bias (bv @ Wo.T + bo).
"""

import numpy as np
import ml_dtypes

import concourse.bass as bass
import concourse.bacc as bacc
import concourse.tile as tile
from concourse import library_config, mybir
from concourse.masks import make_identity

B, Tq, Tk, E, H, D = 2, 1024, 8192, 1024, 16, 64
STRIDE = 16
M = Tk // STRIDE          # 512 keys per head
HPC = 4                   # heads per core
NCORES = 8
F32 = mybir.dt.float32
BF16 = mybir.dt.bfloat16
NPBF16 = ml_dtypes.bfloat16


def build_program():
    nc = bacc.Bacc("TRN2", target_bir_lowering=False)

    # dram tensors already in SBUF layout
    # hsT col = tqt*4096 + e*512 + t ; wq col = pair*1024 + e*128 + j
    hsT = nc.dram_tensor("hsT", [128, 8 * Tq], BF16, kind="ExternalInput")
    wq = nc.dram_tensor("wq", [128, 2048], BF16, kind="ExternalInput")
    wkv = nc.dram_tensor("wkv", [128, 4096], BF16, kind="ExternalInput")
    kvg = nc.dram_tensor("kvg", [HPC, 128, 8 * M], BF16, kind="ExternalInput")
    wo = nc.dram_tensor("wo", [128, 2048], BF16, kind="ExternalInput")
    bqp = nc.dram_tensor("bqp", [2, 128, 1], F32, kind="ExternalInput")
    out = nc.dram_tensor("out", [16, 128, 512], BF16, kind="ExternalOutput")

    _dma_engs = [None, None]
    _dma_i = [0]

    def dma(out_ap, in_ap):
        eng = _dma_engs[_dma_i[0] % 2]
        _dma_i[0] += 1
        eng.dma_start(out=out_ap, in_=in_ap)

    with tile.TileContext(nc) as tc:
        _dma_engs[0] = nc.sync
        _dma_engs[1] = nc.gpsimd
        with (
            tc.tile_pool(name="consts", bufs=1) as consts,
            tc.tile_pool(name="expp", bufs=3) as expp,
            tc.tile_pool(name="rowp", bufs=3) as rowp,
            tc.tile_pool(name="pop", bufs=3) as pop,
            tc.tile_pool(name="psA", bufs=3, space="PSUM") as psA,
            tc.tile_pool(name="psB", bufs=2, space="PSUM") as psB,
        ):
            # ---- persistent SBUF tiles -------------------------------------
            hsT_sb = consts.tile([128, 8 * Tq], BF16)
            wq_sb = consts.tile([128, 2048], BF16)
            wkv_sb = consts.tile([128, 4096], BF16)
            wo_sb = consts.tile([128, 2048], BF16)
            kvg_sb = [consts.tile([128, 8 * M], BF16, name=f"kvg{h}")
                      for h in range(HPC)]
            kv_sb = [consts.tile([128, M], BF16, name=f"kv{h}")
                     for h in range(HPC)]
            vgp_sb = [consts.tile([128, 4, D + 1], BF16, name=f"vgp{h}")
                      for h in range(HPC)]
            qt_sb = [consts.tile([D, Tq], BF16, name=f"qt{h}")
                     for h in range(HPC)]
            outT_sb = [consts.tile([128, Tq], BF16, name=f"outT{dd}")
                       for dd in range(2)]
            bq_sb = [consts.tile([128, 1], F32, name=f"bq{p}") for p in range(2)]

            identf = consts.tile([128, 128], F32)
            make_identity(nc, identf)
            ident = consts.tile([128, 128], BF16)
            nc.vector.tensor_copy(ident[:], identf[:])
            for h in range(HPC):
                nc.vector.memset(vgp_sb[h][:, :, D:D + 1], 1.0)
            # dummy exp so the ACT table load happens during DMA warmup,
            # not on the first real exp of the critical path
            actwarm = consts.tile([1, 1], F32)
            nc.vector.memset(actwarm[:], 0.0)
            nc.scalar.activation(actwarm[:], actwarm[:],
                                 mybir.ActivationFunctionType.Exp)

            # ---- input DMAs, two waves ------------------------------------
            # Wave 1 (phase1 + head0) gets the queues to itself so its bytes
            # land early; later waves are gated behind it via tiny gpsimd
            # reads (DMA queues round-robin between all enqueued transfers,
            # so an ungated bulk enqueue starves the critical path).
            gate_sb = consts.tile([1, 8], BF16)
            _gate_i = [0]

            def gate(sl):
                # 4-byte SBUF->SBUF DMA on sync whose read-dep stalls sync
                # (and thus all later sync dispatches) until `sl`'s writer
                # DMA has landed.
                i = _gate_i[0]
                _gate_i[0] += 1
                nc.sync.dma_start(out=gate_sb[:, i:i + 1], in_=sl)

            # wave 1 (ungated): phase-1 data + head-0 K/V
            dma(wq_sb[:, 0:1024], wq[:, 0:1024])                   # sync
            dma(hsT_sb[:, 0:2048], hsT[:, 0:2048])                 # gpsimd
            dma(hsT_sb[:, 2048:4096], hsT[:, 2048:4096])           # sync
            dma(wq_sb[:, 1024:2048], wq[:, 1024:2048])             # gpsimd
            dma(kvg_sb[0][:, 0:2048], kvg[0][:, 0:2048])           # sync
            dma(kvg_sb[0][:, 2048:4096], kvg[0][:, 2048:4096])     # gpsimd
            dma(hsT_sb[:, 4096:6144], hsT[:, 4096:6144])           # sync
            dma(hsT_sb[:, 6144:8192], hsT[:, 6144:8192])           # gpsimd
            dma(wkv_sb[:, 0:1024], wkv[:, 0:1024])                 # sync
            dma(bq_sb[0][:], bqp[0])
            dma(bq_sb[1][:], bqp[1])
            # waves 2..5 gated per head so each head's bytes land in order
            gate(kvg_sb[0][127:128, 4095:4096])
            gate(hsT_sb[127:128, 8191:8192])
            nc.sync.dma_start(out=kvg_sb[1][:, 0:2048],
                              in_=kvg[1][:, 0:2048])
            nc.sync.dma_start(out=kvg_sb[1][:, 2048:4096],
                              in_=kvg[1][:, 2048:4096])
            nc.sync.dma_start(out=wkv_sb[:, 1024:2048],
                              in_=wkv[:, 1024:2048])
            gate(kvg_sb[1][127:128, 4095:4096])
            nc.sync.dma_start(out=kvg_sb[2][:, 0:2048],
                              in_=kvg[2][:, 0:2048])
            nc.sync.dma_start(out=kvg_sb[2][:, 2048:4096],
                              in_=kvg[2][:, 2048:4096])
            nc.sync.dma_start(out=wkv_sb[:, 2048:3072],
                              in_=wkv[:, 2048:3072])
            gate(kvg_sb[2][127:128, 4095:4096])
            nc.sync.dma_start(out=kvg_sb[3][:, 0:2048],
                              in_=kvg[3][:, 0:2048])
            nc.sync.dma_start(out=kvg_sb[3][:, 2048:4096],
                              in_=kvg[3][:, 2048:4096])
            nc.sync.dma_start(out=wkv_sb[:, 3072:4096],
                              in_=wkv[:, 3072:4096])
            gate(kvg_sb[3][127:128, 4095:4096])
            nc.sync.dma_start(out=wo_sb[:, 0:1024], in_=wo[:, 0:1024])
            nc.sync.dma_start(out=wo_sb[:, 1024:2048],
                              in_=wo[:, 1024:2048])

            # ---- stage closures (issued in an interleaved order so the
            # in-order PE queue never head-of-line blocks on a late DMA) ----
            def ph1(pair, tqt):
                ps_qt = psA.tile([128, 2, 512], F32, tag="A", name="ps_qt")
                for e in range(8):
                    nc.tensor.matmul(
                        ps_qt[:, 0, :],
                        wq_sb[:, pair * 1024 + e * 128:
                              pair * 1024 + (e + 1) * 128],
                        hsT_sb[:, tqt * 4096 + e * 512:
                               tqt * 4096 + (e + 1) * 512],
                        start=(e == 0), stop=(e == 7))
                for sub in range(2):
                    h = 2 * pair + sub
                    nc.vector.tensor_scalar_add(
                        qt_sb[h][:, tqt * 512: tqt * 512 + 512],
                        ps_qt[sub * 64:(sub + 1) * 64, 0, :],
                        bq_sb[pair][sub * 64:(sub + 1) * 64, 0:1])

            def p2a(h):
                ps_kv = psA.tile([128, 2, 512], F32, tag="A", name="ps_kv")
                for e in range(8):
                    nc.tensor.matmul(
                        ps_kv[:, 0, :],
                        wkv_sb[:, (h * 8 + e) * 128:(h * 8 + e + 1) * 128],
                        kvg_sb[h][:, e * M:(e + 1) * M],
                        start=(e == 0), stop=(e == 7))
                nc.scalar.copy(kv_sb[h][:], ps_kv[:, 0, :])
                ps_vt = psB.tile([128, 4, D], BF16, tag="B", name="ps_vt")
                for mc in range(4):
                    nc.tensor.transpose(
                        ps_vt[:, mc, :],
                        kv_sb[h][64:128, mc * 128:(mc + 1) * 128],
                        ident[64:128, 64:128])
                nc.vector.tensor_copy(vgp_sb[h][:, :, 0:D], ps_vt[:])

            pending = []

            def flush():
                while pending:
                    pending.pop(0)()

            def sc_part(tqt, h):
                expT = expp.tile([128, 4, 512], BF16, tag="expT", name="expT")
                for mcp in range(2):
                    ps_sc = psA.tile([128, 2, 512], F32, tag="A", name="ps_sc")
                    for sub in range(2):
                        mc = 2 * mcp + sub
                        nc.tensor.matmul(
                            ps_sc[:, sub, :],
                            kv_sb[h][0:D, mc * 128:(mc + 1) * 128],
                            qt_sb[h][:, tqt * 512: tqt * 512 + 512],
                            start=True, stop=True)
                    nc.scalar.activation(
                        expT[:, 2 * mcp:2 * mcp + 2, :],
                        ps_sc[:],
                        mybir.ActivationFunctionType.Exp)
                return expT

            def av_part(tqt, h, expT):
                if pending:
                    pending.pop(0)()
                ps_ov = psB.tile([128, 512], F32, tag="B", name="ps_ov")
                for mc in range(4):
                    nc.tensor.matmul(
                        ps_ov[0:D + 1, :],
                        vgp_sb[h][:, mc, :],
                        expT[:, mc, :],
                        start=(mc == 0), stop=(mc == 3))
                # rinv = 1/Z on DVE (fast approx), broadcast on GpSimd.
                # Z must reach SBUF partition 0 via a native op first: the
                # custom-DVE recip misreads partition-shifted PSUM on HW.
                zrow = rowp.tile([1, 512], F32, tag="zrow", name="zrow")
                nc.vector.tensor_copy(zrow[:], ps_ov[D:D + 1, :])
                rinv = rowp.tile([1, 512], F32, tag="rinv", name="rinv")
                nc.vector.reciprocal_approx_fast(rinv[:], zrow[:])
                rinv_b = rowp.tile([D, 512], F32, tag="rinv_b", name="rinv_b")
                nc.gpsimd.partition_broadcast(rinv_b[:], rinv[:])

                # defer the normalize-mult by one stage so the vector queue
                # doesn't stall waiting on the gpsimd broadcast
                def _mult(tqt=tqt, h=h, ps_ov=ps_ov, rinv_b=rinv_b):
                    nc.vector.tensor_tensor(
                        outT_sb[h // 2][(h % 2) * D:(h % 2 + 1) * D,
                                        tqt * 512: tqt * 512 + 512],
                        ps_ov[0:D, :],
                        rinv_b[:],
                        op=mybir.AluOpType.mult)
                pending.append(_mult)

            def po(t8):
                flush()
                ps_po = psA.tile([128, 2, 512], F32, tag="A", name="ps_po")
                for eot in range(2):
                    for dd in range(2):
                        nc.tensor.matmul(
                            ps_po[:, eot, :],
                            outT_sb[dd][:, t8 * 128:(t8 + 1) * 128],
                            wo_sb[:, dd * E + eot * 512:
                                  dd * E + eot * 512 + 512],
                            start=(dd == 0), stop=(dd == 1))
                po_sb = pop.tile([128, 2, 512], BF16, tag="po", name="po_sb")
                nc.scalar.copy(po_sb[:, 0, :], ps_po[:, 0, :])
                nc.vector.tensor_copy(po_sb[:, 1, :], ps_po[:, 1, :])
                dma(out[t8 * 2], po_sb[:, 0, :])
                dma(out[t8 * 2 + 1], po_sb[:, 1, :])

            ph1(0, 0)
            ph1(1, 0)
            ph1(0, 1)
            ph1(1, 1)
            p2a(0)
            p2a(1)
            e00 = sc_part(0, 0)
            av_part(0, 0, e00)
            e01 = sc_part(0, 1)
            av_part(0, 1, e01)
            p2a(2)
            e02 = sc_part(0, 2)
            av_part(0, 2, e02)
            p2a(3)
            e03 = sc_part(0, 3)
            av_part(0, 3, e03)
            e10 = sc_part(1, 0)
            av_part(1, 0, e10)
            po(0)
            po(1)
            e11 = sc_part(1, 1)
            av_part(1, 1, e11)
            po(2)
            po(3)
            e12 = sc_part(1, 2)
            av_part(1, 2, e12)
            e13 = sc_part(1, 3)
            av_part(1, 3, e13)
            for t8 in range(4, 8):
                po(t8)

    nc.compile()
    return nc


_NC = None


def _get_nc():
    global _NC
    if _NC is None:
        _NC = build_program()
    return _NC


def shard_inputs(hidden_states, key_value_states, Wq, bq, Wk, bk, Wv, bv, Wo,
                 bo, stride):
    stride = int(stride)
    assert stride == STRIDE
    scale = float(D) ** -0.5
    bf = lambda a: np.ascontiguousarray(a).astype(NPBF16)
    in_maps = []
    for c in range(NCORES):
        b, g = divmod(c, 4)
        h0 = g * HPC
        r0, r1 = h0 * D, (h0 + HPC) * D
        # hsT [128, 8192]: col tqt*4096+e*512+t = hs[b][tqt*512+t, e*128+p]
        hsT_c = hidden_states[b].T.reshape(8, 128, 2, 512) \
            .transpose(1, 2, 0, 3).reshape(128, 8 * Tq)
        # wq [128, 2048]: col pair*1024+e*128+j = Wq_s[pair*128+j, e*128+p]
        Wqs = (Wq[r0:r1, :] * scale)
        wq_c = Wqs.T.reshape(8, 128, 2, 128).transpose(1, 2, 0, 3) \
            .reshape(128, 2048)
        # wkv [128, 4096]: col (h*8+e)*128+j: j<64 Wk, j>=64 Wv
        K3 = Wk[r0:r1, :].reshape(HPC, D, E)
        V3 = Wv[r0:r1, :].reshape(HPC, D, E)
        C = np.concatenate([K3, V3], axis=1)          # [4, 128, 1024]
        wkv_c = C.reshape(4, 128, 8, 128).transpose(3, 0, 2, 1) \
            .reshape(128, 4096)
        # kvg [4, 128, 4096]: col e*512+m = kv[b, h0+h+16m, e*128+p]
        kvg_c = np.empty((HPC, 128, 8 * M), NPBF16)
        for hl in range(HPC):
            R = key_value_states[b, (h0 + hl)::STRIDE, :]     # [512, 1024]
            kvg_c[hl] = bf(R.T.reshape(8, 128, M).transpose(1, 0, 2)
                           .reshape(128, 8 * M))
        # wo [128, 2048]: col dd*1024+n = Wo[n, r0+dd*128+p]
        wo_c = Wo[:, r0:r1].T.reshape(2, 128, E).transpose(1, 0, 2) \
            .reshape(128, 2048)
        bqp_c = (bq[r0:r1] * scale).astype(np.float32).reshape(2, 128, 1)
        in_maps.append({
            "hsT": bf(hsT_c),
            "wq": bf(wq_c),
            "wkv": bf(wkv_c),
            "kvg": np.ascontiguousarray(kvg_c),
            "wo": bf(wo_c),
            "bqp": bqp_c,
        })
    return in_maps


def combine_outputs(results, Wv, bv, Wo, bo):
    final_bias = (bv @ Wo.T + bo).astype(np.float32)  # [E]
    out = np.zeros((B, Tq, E), np.float32)
    for c in range(NCORES):
        b = c // 4
        # out dram [16, 128, 512]: chunk t8*2+eot
        o = results[c]["out"].astype(np.float32) \
            .reshape(8, 2, 128, 512).transpose(0, 2, 1, 3).reshape(Tq, E)
        out[b] += o
    out += final_bias[None, None, :]
    return out


def kernel(hidden_states, key_value_states, Wq, bq, Wk, bk, Wv, bv, Wo, bo,
           stride, _trace=False, _trace_kwargs=None):
    from concourse.bass_utils import run_bass_kernel_spmd

    args = [np.asarray(x, np.float32) for x in
            (hidden_states, key_value_states, Wq, bq, Wk, bk, Wv, bv, Wo, bo)]
    (hidden_states, key_value_states, Wq, bq, Wk, bk, Wv, bv, Wo, bo) = args
    in_maps = shard_inputs(hidden_states, key_value_states, Wq, bq, Wk, bk,
                           Wv, bv, Wo, bo, stride)
    nc = _get_nc()
    res = run_bass_kernel_spmd(
        nc, in_maps, list(range(NCORES)),
        trace=_trace, **(_trace_kwargs or {}))
    out = combine_outputs(res.results, Wv, bv, Wo, bo)
    kernel.last_run = res
    return out
